# revision 1
# baseline (speedup 1.0000x reference)
"""CLIP attention (B=4, S=2048, E=1024, H=16, D=64) on 8 Trainium2 cores.

Sharding: core c handles batch b = c // 2 and heads [ (c%2)*8, (c%2)*8+8 ).
Each core computes its 8 heads' attention plus its partial output
projection (contraction over its 512 local context dims); the host sums
the two partials per batch and adds the output bias.

Per-core dataflow (all activations stored transposed, [feature, seq]):
  hT [E, S]            <- host-pretransposed hidden_states[b], bf16
  qT, kT [512, S]      =  Wq_loc @ hT (+bias, query pre-scaled)   on PE
  v    [S, 512]        =  hT.T @ Wv_loc.T (+bias via bcast add), stored
                          as v_ext tiles [128, 8*65] with a ones column
                          per head (fused softmax denominator)
  ST   [k, q]          =  kT_h.T-slices @ qT_h  (scores, transposed;
                          two heads packed in PE row groups 0-63/64-127)
  P^T  = exp(ST)       on ACT, PSUM -> SBUF bf16 (no max subtraction:
                          scores ~ N(0,1), exp is safe in fp32)
  outT_ext [65, q]     =  V_ext.T @ P^T accumulated over k tiles; row 64
                          is the softmax denominator (ones column)
  CT   [512, S]        =  outT * (1/denom) broadcast  (DVE reciprocal,
                          PE ones-matmul partition broadcast, DVE mul)
  outT_partial [E, S]  =  Wo_loc^T-slices @ CT  -> DRAM fp32

Scheduling: q/k projection chains for head-pair p+1 and finished
s-chunks' output projections are interleaved into pair p's attention
sweeps so the ACT engine (softmax exp, the bottleneck) starts ~20us in
and stays saturated; each sweep's softmax normalization is deferred into
the next sweep's slack.
"""

import numpy as np

B, S, E = 4, 2048, 1024
H, D = 16, 64
SCALE = D ** -0.5
NCORES = 8
HLOC = 8            # heads per core
CLOC = HLOC * D     # 512 local context dims
NHP = HLOC // 2     # 4 head pairs
SC = 512            # seq chunk (matmul moving free dim)
NQC = S // SC       # 4
KT = 128            # k tile rows
NKT = S // KT       # 16
NE = E // 128       # 8 contraction chunks for projections
VW = D + 1          # 65: v columns + fused ones column

_CACHE = {}


def _get_deps():
    import sys
    if "/opt/trn_rl_repo" not in sys.path:
        sys.path.insert(0, "/opt/trn_rl_repo")
    import concourse.bass as bass
    import concourse.mybir as mybir
    import concourse.tile as tile
    return bass, mybir, tile


def _fix_multi_waits(nc, mybir):
    """walrus encodes at most ONE semaphore wait per TPB engine
    instruction. Move surplus waits onto a same-engine Drain inserted just
    before the offending instruction (Drains accept many waits)."""
    for f in nc.m.functions:
        for bb in f.blocks:
            ins = bb.instructions
            if not any(i.sync_info and len(i.sync_info.on_wait) > 1
                       for i in ins):
                continue
            out = []
            for i in ins:
                if i.sync_info and len(i.sync_info.on_wait) > 1:
                    w = list(i.sync_info.on_wait)
                    # a wait on the instruction's OWN processor semaphore
                    # is implied by that processor's FIFO order - drop it
                    own = {u.ant_name for u in i.sync_info.on_update}
                    w2 = [x for x in w if x.ant_name not in own]
                    if not w2:
                        w2 = w[-1:]
                    for j, wj in enumerate(w2[:-1]):
                        d = mybir.InstDrain(
                            name=f"{i.name}_wj{j}", ins=[], outs=[],
                            bass_is_fusable=False)
                        d.engine = i.engine
                        d.sync_info = mybir.SyncInfo(on_wait=[wj], on_update=[])
                        out.append(d)
                    i.sync_info = mybir.SyncInfo(
                        on_wait=w2[-1:], on_update=list(i.sync_info.on_update))
                out.append(i)
            bb.instructions = out


def build_program(fix_waits=True, reps=1):
    """Build the single-core Bass/Tile program (same program on all cores).

    fix_waits: apply the walrus 1-wait-per-instruction fixup (required for
    hardware NEFF compile; CoreSim's race detector can't digest the
    inserted drains, so sim runs pass False)."""
    bass, mybir, tile = _get_deps()
    from contextlib import ExitStack

    f32 = mybir.dt.float32
    bf16 = mybir.dt.bfloat16
    EXP = mybir.ActivationFunctionType.Exp

    nc = bass.Bass()

    hT_d = nc.declare_dram_parameter("hT", [E, S], bf16, isOutput=False)
    wqT_d = nc.declare_dram_parameter("wqT", [E, CLOC], bf16, isOutput=False)
    wkT_d = nc.declare_dram_parameter("wkT", [E, CLOC], bf16, isOutput=False)
    wvT_d = nc.declare_dram_parameter("wvT", [E, CLOC], bf16, isOutput=False)
    woT_d = nc.declare_dram_parameter("woT", [CLOC, E], bf16, isOutput=False)
    bq_d = nc.declare_dram_parameter("bq", [CLOC], f32, isOutput=False)
    bk_d = nc.declare_dram_parameter("bk", [CLOC], f32, isOutput=False)
    bv_d = nc.declare_dram_parameter("bv", [CLOC], f32, isOutput=False)
    outT_d = nc.declare_dram_parameter("outT", [E, S], f32, isOutput=True)

    add = mybir.AluOpType.add
    mult = mybir.AluOpType.mult

    with tile.TileContext(nc) as tc, ExitStack() as ctx:
        sb = ctx.enter_context(tc.tile_pool(name="persist", bufs=1))

        # ---- persistent SBUF tiles ----
        h_sb = [sb.tile([128, S], bf16, name=f"h{e}", tag=f"h{e}") for e in range(NE)]
        wq_sb = [sb.tile([128, CLOC], bf16, name=f"wq{e}", tag=f"wq{e}") for e in range(NE)]
        wk_sb = [sb.tile([128, CLOC], bf16, name=f"wk{e}", tag=f"wk{e}") for e in range(NE)]
        wv_sb = [sb.tile([128, CLOC], bf16, name=f"wv{e}", tag=f"wv{e}") for e in range(NE)]
        wo_sb = [sb.tile([128, E], bf16, name=f"wo{c}", tag=f"wo{c}") for c in range(4)]
        qT_sb = [sb.tile([128, S], bf16, name=f"qT{p}", tag=f"qT{p}") for p in range(NHP)]
        kT_sb = [sb.tile([128, S], bf16, name=f"kT{p}", tag=f"kT{p}") for p in range(NHP)]
        vx_sb = [sb.tile([128, HLOC * VW], bf16, name=f"vx{t}", tag=f"vx{t}") for t in range(NKT)]
        ct_sb = [sb.tile([128, S], bf16, name=f"ct{p}", tag=f"ct{p}") for p in range(NHP)]
        bq_sb = sb.tile([128, 4], f32, name="bq_sb", tag="bq_sb")
        bk_sb = sb.tile([128, 4], f32, name="bk_sb", tag="bk_sb")
        bv_sb = sb.tile([1, CLOC], f32, name="bv_sb", tag="bv_sb")
        bvb_sb = sb.tile([128, CLOC], f32, name="bvb_sb", tag="bvb_sb")
        ones1 = sb.tile([1, 128], f32, name="ones1", tag="ones1")
        ones64 = sb.tile([1, 64], bf16, name="ones64", tag="ones64")

        # ---- input DMAs, ordered by first use, 128 KB chunks so the
        # 8 hardware DMA queues stream them in parallel ----
        nc.sync.dma_start(out=bv_sb[:], in_=bv_d[:])
        for dtile in range(4):
            r = slice(dtile * 128, (dtile + 1) * 128)
            nc.sync.dma_start(out=bq_sb[:, dtile:dtile + 1], in_=bq_d[r])
            nc.sync.dma_start(out=bk_sb[:, dtile:dtile + 1], in_=bk_d[r])
        for e in range(NE):
            r = slice(e * 128, (e + 1) * 128)
            nc.sync.dma_start(out=wv_sb[e][:], in_=wvT_d[r, :])
        scol0 = slice(0, SC)
        for e in range(NE):
            r = slice(e * 128, (e + 1) * 128)
            nc.sync.dma_start(out=h_sb[e][:, scol0], in_=hT_d[r, scol0])
        for e in range(NE):
            r = slice(e * 128, (e + 1) * 128)
            nc.sync.dma_start(out=wq_sb[e][:], in_=wqT_d[r, :])
        for e in range(NE):
            r = slice(e * 128, (e + 1) * 128)
            nc.sync.dma_start(out=wk_sb[e][:], in_=wkT_d[r, :])
        for sc in range(1, NQC):
            scol = slice(sc * SC, (sc + 1) * SC)
            for e in range(NE):
                r = slice(e * 128, (e + 1) * 128)
                nc.sync.dma_start(out=h_sb[e][:, scol], in_=hT_d[r, scol])
        for c in range(4):
            nc.sync.dma_start(out=wo_sb[c][:], in_=woT_d[c * 128:(c + 1) * 128, :])
        nc.vector.memset(ones1[:], 1.0)
        nc.vector.memset(ones64[:], 1.0)

        # ones columns of v_ext (softmax denominator fusion), set once
        for t in range(NKT):
            for h in range(HLOC):
                nc.vector.memset(vx_sb[t][:, h * VW + D:h * VW + D + 1], 1.0)

        for _rep in range(reps):
            with tc.tile_pool(name="ppj", bufs=2, space="PSUM") as ppj, \
                 tc.tile_pool(name="stp", bufs=2, space="PSUM") as stp, \
                 tc.tile_pool(name="exs", bufs=4) as exs, \
                 tc.tile_pool(name="nrm", bufs=3) as nrm, \
                 tc.tile_pool(name="ost", bufs=4) as ost:
                # ppj's "pps" tag (2 slots x 2 banks, sized by the av pair
                # tile) carries: AV accumulation pairs, interleaved q/k
                # projection chains, and out-projection accumulators.
                # stp's "st" tag (2 slots x 2 banks): score tiles + the
                # 1/denom broadcast.

                _chain_ps = {}

                def qk_chain(w_sb, b_sb, dst_sb, p, scnk, half=None):
                    # half=0/1 emits the first/second 4 accumulation steps
                    # (smaller PE bursts so the exp pipeline never drains);
                    # half=None emits the whole chain
                    dcol = slice(p * 128, (p + 1) * 128)
                    scol = slice(scnk * SC, (scnk + 1) * SC)
                    if half in (None, 0):
                        ps = ppj.tile([128, SC], f32, name="qkps", tag="pps")
                        _chain_ps[(p, scnk, dst_sb is qT_sb)] = ps
                    else:
                        ps = _chain_ps.pop((p, scnk, dst_sb is qT_sb))
                    es = (range(NE) if half is None
                          else range(half * NE // 2, (half + 1) * NE // 2))
                    for e in es:
                        nc.tensor.matmul(
                            ps[:], w_sb[e][:, dcol], h_sb[e][:, scol],
                            start=(e == 0), stop=(e == NE - 1),
                            skip_group_check=True)
                    if half in (None, 1):
                        nc.vector.tensor_scalar(
                            dst_sb[p][:, scol], ps[:], b_sb[:, p:p + 1],
                            None, op0=add)

                def v_chain(st):
                    ps = ppj.tile([128, CLOC], f32, name="vps", tag="pps")
                    for e in range(NE):
                        nc.tensor.matmul(
                            ps[:], h_sb[e][:, st * 128:(st + 1) * 128], wv_sb[e][:],
                            start=(e == 0), stop=(e == NE - 1))
                    nc.vector.tensor_tensor(
                        vx_sb[st][:].rearrange("p (h w) -> p h w", w=VW)[:, :, 0:D],
                        ps[:].rearrange("p (h w) -> p h w", w=D),
                        bvb_sb[:].rearrange("p (h w) -> p h w", w=D),
                        op=add)

                def out_proj_tile(scnk, et):
                    scol = slice(scnk * SC, (scnk + 1) * SC)
                    erow = slice(et * 128, (et + 1) * 128)
                    ps = ppj.tile([128, SC], f32, name="ops", tag="pps")
                    for c in range(4):
                        nc.tensor.matmul(
                            ps[:], wo_sb[c][:, erow], ct_sb[c][:, scol],
                            start=(c == 0), stop=(c == 3))
                    ot = ost.tile([128, SC], f32, name="ot", tag="ot")
                    nc.vector.tensor_copy(ot[:], ps[:])
                    nc.sync.dma_start(out=outT_d[erow, scol], in_=ot[:])

                # head: v-bias broadcast, the minimum projections the
                # first sweep needs (v tiles 0-3, q/k chunk 0 of pair 0);
                # everything else is interleaved into the sweeps
                bb_ps = ppj.tile([128, CLOC], f32, name="bbps", tag="pps")
                nc.tensor.matmul(bb_ps[:], ones1[:], bv_sb[:], start=True,
                                 stop=True)
                nc.vector.tensor_copy(bvb_sb[:], bb_ps[:])
                for st in range(4):
                    v_chain(st)
                qk_chain(wq_sb, bq_sb, qT_sb, 0, 0)
                qk_chain(wk_sb, bk_sb, kT_sb, 0, 0)

                # attention sweeps with PE filler work (q/k chains for the
                # next pair, out-projection for finished s-chunks) spread
                # into each sweep's slack
                chainq = []
                opq = []
                pending_norm = []

                def normalize(av, p, qc):
                    qcol = slice(qc * SC, (qc + 1) * SC)
                    rr = nrm.tile([1, 2 * SC], bf16, name="rr", tag="rr")
                    with nc.allow_low_precision("1/denom in bf16: <=2^-9 rel"):
                        nc.vector.reciprocal(rr[:], av[64:65, :])
                    bc = stp.tile([64, 2 * SC], f32, name="bc", tag="st")
                    nc.tensor.matmul(bc[:, 0:SC], ones64[:], rr[0:1, 0:SC],
                                     start=True, stop=True)
                    nc.tensor.matmul(bc[:, SC:2 * SC], ones64[:],
                                     rr[0:1, SC:2 * SC], start=True,
                                     stop=True)
                    rb = nrm.tile([64, 2 * SC], f32, name="rb", tag="rb")
                    nc.vector.tensor_copy(rb[:], bc[:])
                    nc.vector.tensor_tensor(
                        ct_sb[p][0:64, qcol], av[0:64, 0:SC], rb[:, 0:SC],
                        op=mult)
                    nc.vector.tensor_tensor(
                        ct_sb[p][64:128, qcol], av[0:64, SC:2 * SC],
                        rb[:, SC:2 * SC], op=mult)
                    if p == NHP - 1:
                        for et in range(NE):
                            opq.append(
                                (lambda s=qc, e=et: out_proj_tile(s, e)))

                # first sweep's chain work, ordered by first use inside
                # the (p0, qc0) k-tile loop: k-chunks land before their
                # score tiles, v tiles before their AV matmuls
                for s in range(1, NQC):
                    chainq.append(
                        (lambda s=s: qk_chain(wk_sb, bk_sb, kT_sb, 0, s)))
                for st in range(4, NKT):
                    chainq.append((lambda st=st: v_chain(st)))
                # reorder: k1 first, then v4.., k2 after v6, k3 after v9
                chainq = [chainq[0], chainq[3], chainq[4], chainq[5],
                          chainq[1], chainq[6], chainq[7], chainq[8],
                          chainq[2]] + chainq[9:]
                for s in range(1, NQC):
                    chainq.append(
                        (lambda s=s: qk_chain(wq_sb, bq_sb, qT_sb, 0, s)))
                for p in range(NHP):
                    if p < NHP - 1:
                        # half-chains for pair p+1, pulled during pair p's
                        # sweeps (two pulls per projection chunk)
                        for s in range(NQC):
                            for (w, b, d) in ((wq_sb, bq_sb, qT_sb),
                                              (wk_sb, bk_sb, kT_sb)):
                                for hf in (0, 1):
                                    chainq.append(
                                        (lambda w=w, b=b, d=d, s=s, q=p + 1,
                                         hf=hf: qk_chain(w, b, d, q, s, hf)))
                    for qc in range(NQC):
                        qcol = slice(qc * SC, (qc + 1) * SC)
                        av = ppj.tile([VW, 2 * SC], f32, name="av", tag="pps")
                        prev_ex = None
                        for kt in range(NKT + 1):
                            if kt < NKT:
                                kcol = slice(kt * 128, (kt + 1) * 128)
                                st_t = stp.tile([128, 2 * SC], f32, name="st",
                                                tag="st")
                                nc.tensor.matmul(
                                    st_t[:, 0:SC], kT_sb[p][0:64, kcol],
                                    qT_sb[p][0:64, qcol],
                                    start=True, stop=True, tile_position=(0, 0))
                                nc.tensor.matmul(
                                    st_t[:, SC:2 * SC], kT_sb[p][64:128, kcol],
                                    qT_sb[p][64:128, qcol],
                                    start=True, stop=True, tile_position=(64, 0))
                                ex = exs.tile([128, 2 * SC], bf16, name="ex",
                                              tag="ex")
                                nc.scalar.activation(ex[:], st_t[:], EXP)
                            if kt > 0:
                                j = kt - 1
                                for hh in range(2):
                                    h = 2 * p + hh
                                    nc.tensor.matmul(
                                        av[:, hh * SC:(hh + 1) * SC],
                                        vx_sb[j][:, h * VW:h * VW + VW],
                                        prev_ex[:, hh * SC:(hh + 1) * SC],
                                        start=(j == 0), stop=(j == NKT - 1),
                                        skip_group_check=True)
                            if kt < NKT:
                                prev_ex = ex
                            # filler work in the sweep's PE slack. The
                            # very first sweep pulls a chain every k-tile
                            # (it must produce the projections it consumes);
                            # later sweeps pace chains 4 per sweep.
                            if kt == 2 and pending_norm:
                                args = pending_norm.pop(0)
                                normalize(*args)
                            dense = (p == 0 and qc == 0)
                            if (kt >= 0 if dense else kt % 2 == 1) \
                                    and chainq:
                                chainq.pop(0)()
                            elif kt > 0 and opq:
                                opq.pop(0)()
                        # normalize is deferred into the next sweep's
                        # slack so its recip->broadcast chain never blocks
                        # the next sweep's score matmuls
                        pending_norm.append((av, p, qc))
                while pending_norm:
                    normalize(*pending_norm.pop(0))
                # drain remaining out-projection tiles
                while opq:
                    opq.pop(0)()

    if fix_waits:
        _fix_multi_waits(nc, mybir)
    return nc


def make_inputs(hidden_states, Wq, bq, Wk, bk, Wv, bv, Wo, bo):
    """Shard + preprocess the full inputs into 8 per-core input maps.
    Shared pieces (per-batch hidden transpose, per-half weight slices)
    are computed once and referenced by both cores that use them."""
    import ml_dtypes
    bf16 = ml_dtypes.bfloat16
    f32 = np.float32

    hidden_states = np.asarray(hidden_states, f32)
    hT = [np.ascontiguousarray(hidden_states[b].T).astype(bf16)
          for b in range(B)]
    halves = []
    for half in range(2):
        hs = slice(half * CLOC, half * CLOC + CLOC)
        halves.append({
            "wqT": np.ascontiguousarray(
                (np.asarray(Wq, f32)[hs] * SCALE).T).astype(bf16),
            "wkT": np.ascontiguousarray(np.asarray(Wk, f32)[hs].T).astype(bf16),
            "wvT": np.ascontiguousarray(np.asarray(Wv, f32)[hs].T).astype(bf16),
            "woT": np.ascontiguousarray(np.asarray(Wo, f32)[:, hs].T).astype(bf16),
            "bq": np.ascontiguousarray(np.asarray(bq, f32)[hs] * SCALE),
            "bk": np.ascontiguousarray(np.asarray(bk, f32)[hs]),
            "bv": np.ascontiguousarray(np.asarray(bv, f32)[hs]),
        })
    return [{"hT": hT[c // 2], **halves[c % 2]} for c in range(NCORES)]


def gather_output(results, bo):
    out = np.empty((B, S, E), np.float32)
    bo = np.asarray(bo, np.float32)
    for b in range(B):
        acc = results[2 * b]["outT"].astype(np.float32) + \
              results[2 * b + 1]["outT"].astype(np.float32)
        out[b] = acc.T + bo
    return out


def _get_runner():
    """Build the Bass program + jitted 8-core executable once; reuse."""
    if "runner" in _CACHE:
        return _CACHE["runner"]
    _get_deps()
    import jax
    import numpy as np
    from jax.sharding import Mesh, PartitionSpec
    from jax.experimental.shard_map import shard_map
    from concourse import bass2jax, mybir

    bass2jax.install_neuronx_cc_hook()
    nc = build_program()

    partition_name = (nc.partition_id_tensor.name
                      if nc.partition_id_tensor else None)
    in_names, out_names, out_avals = [], [], []
    for alloc in nc.m.functions[0].allocations:
        if not isinstance(alloc, mybir.MemoryLocationSet):
            continue
        name = alloc.memorylocations[0].name
        if alloc.kind == "ExternalInput":
            if name != partition_name:
                in_names.append(name)
        elif alloc.kind == "ExternalOutput":
            out_names.append(name)
            out_avals.append(jax.core.ShapedArray(
                tuple(alloc.tensor_shape), mybir.dt.np(alloc.dtype)))
    n_params = len(in_names)
    all_in_names = in_names + out_names
    if partition_name is not None:
        all_in_names = all_in_names + [partition_name]

    def _body(*args):
        operands = list(args)
        if partition_name is not None:
            operands.append(bass2jax.partition_id_tensor())
        outs = bass2jax._bass_exec_p.bind(
            *operands,
            out_avals=tuple(out_avals),
            in_names=tuple(all_in_names),
            out_names=tuple(out_names),
            lowering_input_output_aliases=(),
            sim_require_finite=True,
            sim_require_nnan=True,
            nc=nc,
        )
        return tuple(outs)

    devices = jax.devices()[:NCORES]
    mesh = Mesh(np.asarray(devices), ("core",))
    n_outs = len(out_avals)
    sharded = jax.jit(
        shard_map(
            _body, mesh=mesh,
            in_specs=(PartitionSpec("core"),) * (n_params + n_outs),
            out_specs=(PartitionSpec("core"),) * n_outs,
            check_rep=False,
        ),
        donate_argnums=tuple(range(n_params, n_params + n_outs)),
        keep_unused=True,
    )

    def run(in_maps):
        concat_in = [
            np.concatenate([np.asarray(in_maps[c][nm]) for c in range(NCORES)],
                           axis=0)
            for nm in in_names
        ]
        concat_zeros = [
            np.zeros((NCORES * a.shape[0], *a.shape[1:]), a.dtype)
            for a in out_avals
        ]
        out_arrs = sharded(*concat_in, *concat_zeros)
        return [
            {nm: np.asarray(out_arrs[i]).reshape(NCORES, *out_avals[i].shape)[c]
             for i, nm in enumerate(out_names)}
            for c in range(NCORES)
        ]

    _CACHE["runner"] = (run, sharded, in_names, out_avals)
    return _CACHE["runner"]


def kernel(hidden_states, Wq, bq, Wk, bk, Wv, bv, Wo, bo):
    run = _get_runner()[0]
    in_maps = make_inputs(hidden_states, Wq, bq, Wk, bk, Wv, bv, Wo, bo)
    results = run(in_maps)
    return gather_output(results, bo)


def bench(in_maps, iters=20, pipeline=True):
    """Time repeated device executions with device-resident inputs and a
    non-donating jit (zeros reused). Returns per-iter seconds."""
    import time
    import numpy as np
    import jax
    from jax.sharding import Mesh, NamedSharding, PartitionSpec
    from jax.experimental.shard_map import shard_map

    run, sharded, in_names, out_avals = _get_runner()

    devices = jax.devices()[:NCORES]
    mesh = Mesh(np.asarray(devices), ("core",))
    sh = NamedSharding(mesh, PartitionSpec("core"))
    concat_in = [
        np.concatenate([np.asarray(in_maps[c][nm]) for c in range(NCORES)], axis=0)
        for nm in in_names
    ]
    dev_in = [jax.device_put(a, sh) for a in concat_in]
    # zeros are donated (consumed) per execution: pre-stage one set per iter
    znp = [np.zeros((NCORES * a.shape[0], *a.shape[1:]), a.dtype)
           for a in out_avals]
    zsets = [[jax.device_put(z, sh) for z in znp] for _ in range(iters + 1)]

    jax.block_until_ready(sharded(*dev_in, *zsets[-1]))  # warm

    if pipeline:
        t0 = time.perf_counter()
        outs = [sharded(*dev_in, *zsets[i]) for i in range(iters)]
        jax.block_until_ready(outs)
        tot = time.perf_counter() - t0
        return [tot / iters] * iters
    ts = []
    for i in range(iters):
        t0 = time.perf_counter()
        jax.block_until_ready(sharded(*dev_in, *zsets[i]))
        ts.append(time.perf_counter() - t0)
    return ts


if __name__ == "__main__":
    rng = np.random.default_rng(0)
    ins = {
        "hidden_states": rng.standard_normal((B, S, E), np.float32),
        "Wq": rng.standard_normal((E, E), np.float32) * E ** -0.5,
        "bq": rng.standard_normal(E).astype(np.float32) * 0.02,
        "Wk": rng.standard_normal((E, E), np.float32) * E ** -0.5,
        "bk": rng.standard_normal(E).astype(np.float32) * 0.02,
        "Wv": rng.standard_normal((E, E), np.float32) * E ** -0.5,
        "bv": rng.standard_normal(E).astype(np.float32) * 0.02,
        "Wo": rng.standard_normal((E, E), np.float32) * E ** -0.5,
        "bo": rng.standard_normal(E).astype(np.float32) * 0.02,
    }
    out = kernel(**ins)
    print(out.shape, out.dtype, np.abs(out).max())



# revision 10
# speedup vs baseline: 1.2655x; 1.2655x over previous
"""CLIP attention (B=4, S=2048, E=1024, H=16, D=64) on 8 Trainium2 cores.

Sharding: core c handles batch b = c // 2 and heads [ (c%2)*8, (c%2)*8+8 ).
Each core computes its 8 heads' attention plus its partial output
projection (contraction over its 512 local context dims); the host sums
the two partials per batch and adds the output bias.

Per-core dataflow (all activations stored transposed, [feature, seq]):
  hT [E, S]            <- host-pretransposed hidden_states[b], bf16
  qT, kT [512, S]      =  Wq_loc @ hT (+bias, query pre-scaled)   on PE
  v    [S, 512]        =  hT.T @ Wv_loc.T (+bias via bcast add), stored
                          as v_ext tiles [128, 8*65] with a ones column
                          per head (fused softmax denominator)
  ST   [k, q]          =  kT_h.T-slices @ qT_h  (scores, transposed;
                          two heads packed in PE row groups 0-63/64-127)
  P^T  = exp(ST)       on ACT, PSUM -> SBUF bf16 (no max subtraction:
                          scores ~ N(0,1), exp is safe in fp32)
  outT_ext [65, q]     =  V_ext.T @ P^T accumulated over k tiles; row 64
                          is the softmax denominator (ones column)
  CT   [512, S]        =  outT * (1/denom) broadcast
  outT_partial [E, S]  =  Wo_loc^T-slices @ CT  -> DRAM fp32

PSUM (8 banks): score tiles 2 slots x 2 banks, av accumulator 1 slot x
2 banks, projection chains 2 slots x 1 bank. The av slot is single-
buffered; av consumption lags exp by ~6 k-tiles so the previous sweep's
normalize (which reads the av slot) completes before this sweep's first
av matmul needs it.

Normalize path (per sweep): the denominator row av[64,:] is copied to
SBUF (DVE), DMA-reshaped [1,1024]->[128,8] so reciprocal runs on 128
DVE lanes instead of 1 (6.5us -> 0.15us), DMA-flattened back, then
broadcast to 64 partitions by two ones-matmuls into projection-pool
banks, copied to SBUF and multiplied into the context (DVE).

Scheduling: per k-tile slot the emission order is score pair -> filler
(next pair's q/k projection chains, finished chunks' out-projections,
front-loaded into slots 0-7) -> lagged av pairs, so a dependency-stalled
av matmul never head-of-line-blocks filler work in the PE queue.
"""

import numpy as np

B, S, E = 4, 2048, 1024
H, D = 16, 64
SCALE = D ** -0.5
NCORES = 8
HLOC = 8            # heads per core
CLOC = HLOC * D     # 512 local context dims
NHP = HLOC // 2     # 4 head pairs
SC = 512            # seq chunk (matmul moving free dim)
NQC = S // SC       # 4
KT = 128            # k tile rows
NKT = S // KT       # 16
NE = E // 128       # 8 contraction chunks for projections
VW = D + 1          # 65: v columns + fused ones column

# av-consumption schedule: slot k of a sweep consumes these exp tiles'
# AV contributions. Lag ~8 slots gives the previous sweep's normalize
# time to release the single av slot; doubled-up late slots clear the
# backlog before the sweep ends.
AV_SCHED = {8: [0], 9: [1], 10: [2], 11: [3, 4], 12: [5, 6],
            13: [7, 8], 14: [9, 10], 15: [11, 12], 16: [13, 14, 15]}

_CACHE = {}


def _get_deps():
    import sys
    if "/opt/trn_rl_repo" not in sys.path:
        sys.path.insert(0, "/opt/trn_rl_repo")
    import concourse.bass as bass
    import concourse.mybir as mybir
    import concourse.tile as tile
    return bass, mybir, tile


def _fix_multi_waits(nc, mybir):
    """walrus encodes at most ONE semaphore wait per TPB engine
    instruction. Move surplus waits onto a same-engine Drain inserted just
    before the offending instruction (Drains accept many waits)."""
    for f in nc.m.functions:
        for bb in f.blocks:
            ins = bb.instructions
            if not any(i.sync_info and len(i.sync_info.on_wait) > 1
                       for i in ins):
                continue
            out = []
            for i in ins:
                if i.sync_info and len(i.sync_info.on_wait) > 1:
                    w = list(i.sync_info.on_wait)
                    # a wait on the instruction's OWN processor semaphore
                    # is implied by that processor's FIFO order - drop it
                    own = {u.ant_name for u in i.sync_info.on_update}
                    w2 = [x for x in w if x.ant_name not in own]
                    if not w2:
                        w2 = w[-1:]
                    for j, wj in enumerate(w2[:-1]):
                        d = mybir.InstDrain(
                            name=f"{i.name}_wj{j}", ins=[], outs=[],
                            bass_is_fusable=False)
                        d.engine = i.engine
                        d.sync_info = mybir.SyncInfo(on_wait=[wj], on_update=[])
                        out.append(d)
                    i.sync_info = mybir.SyncInfo(
                        on_wait=w2[-1:], on_update=list(i.sync_info.on_update))
                out.append(i)
            bb.instructions = out


def build_program(fix_waits=True, reps=1):
    """Build the single-core Bass/Tile program (same program on all cores)."""
    bass, mybir, tile = _get_deps()
    from contextlib import ExitStack

    f32 = mybir.dt.float32
    bf16 = mybir.dt.bfloat16
    EXP = mybir.ActivationFunctionType.Exp

    nc = bass.Bass()

    hT_d = nc.declare_dram_parameter("hT", [E, S], bf16, isOutput=False)
    wqT_d = nc.declare_dram_parameter("wqT", [E, CLOC], bf16, isOutput=False)
    wkT_d = nc.declare_dram_parameter("wkT", [E, CLOC], bf16, isOutput=False)
    wvT_d = nc.declare_dram_parameter("wvT", [E, CLOC], bf16, isOutput=False)
    woT_d = nc.declare_dram_parameter("woT", [CLOC, E], bf16, isOutput=False)
    bq_d = nc.declare_dram_parameter("bq", [CLOC], f32, isOutput=False)
    bk_d = nc.declare_dram_parameter("bk", [CLOC], f32, isOutput=False)
    bv_d = nc.declare_dram_parameter("bv", [CLOC], f32, isOutput=False)
    outT_d = nc.declare_dram_parameter("outT", [E, S], f32, isOutput=True)

    add = mybir.AluOpType.add
    mult = mybir.AluOpType.mult

    with tile.TileContext(nc) as tc, ExitStack() as ctx:
        sb = ctx.enter_context(tc.tile_pool(name="persist", bufs=1))

        # ---- persistent SBUF tiles ----
        h_sb = [sb.tile([128, S], bf16, name=f"h{e}", tag=f"h{e}") for e in range(NE)]
        wq_sb = [sb.tile([128, CLOC], bf16, name=f"wq{e}", tag=f"wq{e}") for e in range(NE)]
        wk_sb = [sb.tile([128, CLOC], bf16, name=f"wk{e}", tag=f"wk{e}") for e in range(NE)]
        wv_sb = [sb.tile([128, CLOC], bf16, name=f"wv{e}", tag=f"wv{e}") for e in range(NE)]
        wo_sb = [sb.tile([128, E], bf16, name=f"wo{c}", tag=f"wo{c}") for c in range(4)]
        qT_sb = [sb.tile([128, S], bf16, name=f"qT{p}", tag=f"qT{p}") for p in range(NHP)]
        kT_sb = [sb.tile([128, S], bf16, name=f"kT{p}", tag=f"kT{p}") for p in range(NHP)]
        vx_sb = [sb.tile([128, HLOC * VW], bf16, name=f"vx{t}", tag=f"vx{t}") for t in range(NKT)]
        ct_sb = [sb.tile([128, S], bf16, name=f"ct{p}", tag=f"ct{p}") for p in range(NHP)]
        bq_sb = sb.tile([128, 4], f32, name="bq_sb", tag="bq_sb")
        bk_sb = sb.tile([128, 4], f32, name="bk_sb", tag="bk_sb")
        bv_sb = sb.tile([1, CLOC], f32, name="bv_sb", tag="bv_sb")
        bvb_sb = sb.tile([128, CLOC], f32, name="bvb_sb", tag="bvb_sb")
        ones1 = sb.tile([1, 128], f32, name="ones1", tag="ones1")
        ones64 = sb.tile([1, 64], bf16, name="ones64", tag="ones64")

        # ---- input DMAs, ordered by first use, 128 KB chunks so the
        # 8 hardware DMA queues stream them in parallel ----
        nc.sync.dma_start(out=bv_sb[:], in_=bv_d[:])
        for dtile in range(4):
            r = slice(dtile * 128, (dtile + 1) * 128)
            nc.sync.dma_start(out=bq_sb[:, dtile:dtile + 1], in_=bq_d[r])
            nc.sync.dma_start(out=bk_sb[:, dtile:dtile + 1], in_=bk_d[r])
        for e in range(NE):
            r = slice(e * 128, (e + 1) * 128)
            nc.sync.dma_start(out=wv_sb[e][:], in_=wvT_d[r, :])
        scol0 = slice(0, SC)
        for e in range(NE):
            r = slice(e * 128, (e + 1) * 128)
            nc.sync.dma_start(out=h_sb[e][:, scol0], in_=hT_d[r, scol0])
        for e in range(NE):
            r = slice(e * 128, (e + 1) * 128)
            nc.sync.dma_start(out=wq_sb[e][:], in_=wqT_d[r, :])
        for e in range(NE):
            r = slice(e * 128, (e + 1) * 128)
            nc.sync.dma_start(out=wk_sb[e][:], in_=wkT_d[r, :])
        for sc in range(1, NQC):
            scol = slice(sc * SC, (sc + 1) * SC)
            for e in range(NE):
                r = slice(e * 128, (e + 1) * 128)
                nc.sync.dma_start(out=h_sb[e][:, scol], in_=hT_d[r, scol])
        for c in range(4):
            nc.sync.dma_start(out=wo_sb[c][:], in_=woT_d[c * 128:(c + 1) * 128, :])
        nc.vector.memset(ones1[:], 1.0)
        nc.vector.memset(ones64[:], 1.0)

        # ones columns of v_ext (softmax denominator fusion), set once
        for t in range(NKT):
            for h in range(HLOC):
                nc.vector.memset(vx_sb[t][:, h * VW + D:h * VW + D + 1], 1.0)

        # ---- persistent pools (live across reps) ----
        stp = ctx.enter_context(tc.tile_pool(name="stp", bufs=2, space="PSUM"))
        avp = ctx.enter_context(tc.tile_pool(name="avp", bufs=1, space="PSUM"))
        pjp = ctx.enter_context(tc.tile_pool(name="pjp", bufs=2, space="PSUM"))
        exs = ctx.enter_context(tc.tile_pool(name="exs", bufs=10))
        nrm = ctx.enter_context(tc.tile_pool(name="nrm", bufs=2))
        ost = ctx.enter_context(tc.tile_pool(name="ost", bufs=4))

        _chain_ps = {}
        opq = []            # deferred out-projection tiles (persist across reps)

        def qk_chain(w_sb, b_sb, dst_sb, p, scnk, half=None):
            # half=0/1 emits the first/second 4 accumulation steps
            # (smaller PE bursts); half=None emits the whole chain
            dcol = slice(p * 128, (p + 1) * 128)
            scol = slice(scnk * SC, (scnk + 1) * SC)
            if half in (None, 0):
                ps = pjp.tile([128, SC], f32, name="qkps", tag="pj")
                _chain_ps[(p, scnk, dst_sb is qT_sb)] = ps
            else:
                ps = _chain_ps.pop((p, scnk, dst_sb is qT_sb))
            es = (range(NE) if half is None
                  else range(half * NE // 2, (half + 1) * NE // 2))
            for e in es:
                nc.tensor.matmul(
                    ps[:], w_sb[e][:, dcol], h_sb[e][:, scol],
                    start=(e == 0), stop=(e == NE - 1),
                    skip_group_check=True)
            if half in (None, 1):
                nc.vector.tensor_scalar(
                    dst_sb[p][:, scol], ps[:], b_sb[:, p:p + 1],
                    None, op0=add)

        def v_chain(st):
            ps = pjp.tile([128, CLOC], f32, name="vps", tag="pj")
            for e in range(NE):
                nc.tensor.matmul(
                    ps[:], h_sb[e][:, st * 128:(st + 1) * 128], wv_sb[e][:],
                    start=(e == 0), stop=(e == NE - 1))
            nc.vector.tensor_tensor(
                vx_sb[st][:].rearrange("p (h w) -> p h w", w=VW)[:, :, 0:D],
                ps[:].rearrange("p (h w) -> p h w", w=D),
                bvb_sb[:].rearrange("p (h w) -> p h w", w=D),
                op=add)

        def out_proj_tile(scnk, et):
            scol = slice(scnk * SC, (scnk + 1) * SC)
            erow = slice(et * 128, (et + 1) * 128)
            ps = pjp.tile([128, SC], f32, name="ops", tag="pj")
            for c in range(4):
                nc.tensor.matmul(
                    ps[:], wo_sb[c][:, erow], ct_sb[c][:, scol],
                    start=(c == 0), stop=(c == 3))
            ot = ost.tile([128, SC], f32, name="ot", tag="ot")
            nc.vector.tensor_copy(ot[:], ps[:])
            nc.sync.dma_start(out=outT_d[erow, scol], in_=ot[:])

        def norm_pre(av):
            """Compute rrf = 1/denom [1, 2SC] bf16: DVE copy of the
            denominator row, DMA reshape to 128 lanes, DVE reciprocal,
            DMA flatten back."""
            dn = nrm.tile([1, 2 * SC], f32, name="dn", tag="dn")
            nc.vector.tensor_copy(dn[:], av[64:65, :])
            dn128 = nrm.tile([128, 8], f32, name="dn128", tag="dn128")
            nc.sync.dma_start(out=dn128[:], in_=dn[:])
            rr128 = nrm.tile([128, 8], bf16, name="rr128", tag="rr128")
            with nc.allow_low_precision("1/denom in bf16: <=2^-9 rel"):
                nc.vector.reciprocal(rr128[:], dn128[:])
            rrf = nrm.tile([1, 2 * SC], bf16, name="rrf", tag="rrf")
            nc.sync.dma_start(out=rrf[:], in_=rr128[:])
            return rrf

        def norm_fin(av, rrf, p, qc):
            """Broadcast 1/denom to 64 partitions (two ones-matmuls into
            projection-pool banks) and multiply into the context."""
            qcol = slice(qc * SC, (qc + 1) * SC)
            rb = nrm.tile([64, 2 * SC], f32, name="rb", tag="rb")
            for hh in range(2):
                cols = slice(hh * SC, (hh + 1) * SC)
                bc = pjp.tile([64, SC], f32, name="bc", tag="pj")
                nc.tensor.matmul(bc[:], ones64[:], rrf[0:1, cols],
                                 start=True, stop=True)
                nc.vector.tensor_copy(rb[:, cols], bc[:])
            nc.vector.tensor_tensor(
                ct_sb[p][0:64, qcol], av[0:64, 0:SC], rb[:, 0:SC],
                op=mult)
            nc.vector.tensor_tensor(
                ct_sb[p][64:128, qcol], av[0:64, SC:2 * SC],
                rb[:, SC:2 * SC], op=mult)
            if p == NHP - 1:
                for et in range(NE):
                    opq.append((lambda s=qc, e=et: out_proj_tile(s, e)))

        for _rep in range(reps):
            # head: v-bias broadcast, the minimum projections the
            # first sweep needs (v tiles 0-3, q/k chunk 0 of pair 0)
            bb_ps = pjp.tile([128, CLOC], f32, name="bbps", tag="pj")
            nc.tensor.matmul(bb_ps[:], ones1[:], bv_sb[:], start=True,
                             stop=True)
            nc.vector.tensor_copy(bvb_sb[:], bb_ps[:])
            for st in range(4):
                v_chain(st)
            qk_chain(wq_sb, bq_sb, qT_sb, 0, 0)
            qk_chain(wk_sb, bk_sb, kT_sb, 0, 0)

            chainq = []
            # first sweep's chain work, ordered by first use inside
            # the (p0, qc0) k-tile loop: k-chunks land before their
            # score tiles, v tiles before their AV matmuls
            for s in range(1, NQC):
                chainq.append(
                    (lambda s=s: qk_chain(wk_sb, bk_sb, kT_sb, 0, s)))
            for st in range(4, NKT):
                chainq.append((lambda st=st: v_chain(st)))
            # reorder: k1 first, then v4.., k2 after v6, k3 after v9
            chainq = [chainq[0], chainq[3], chainq[4], chainq[5],
                      chainq[1], chainq[6], chainq[7], chainq[8],
                      chainq[2]] + chainq[9:]
            for s in range(1, NQC):
                chainq.append(
                    (lambda s=s: qk_chain(wq_sb, bq_sb, qT_sb, 0, s)))

            pending_fin = []
            for p in range(NHP):
                if p < NHP - 1:
                    # half-chains for pair p+1, pulled during pair p's
                    # sweeps (two pulls per projection chunk)
                    for s in range(NQC):
                        for (w, b, d) in ((wq_sb, bq_sb, qT_sb),
                                          (wk_sb, bk_sb, kT_sb)):
                            for hf in (0, 1):
                                chainq.append(
                                    (lambda w=w, b=b, d=d, s=s, q=p + 1,
                                     hf=hf: qk_chain(w, b, d, q, s, hf)))
                for qc in range(NQC):
                    qcol = slice(qc * SC, (qc + 1) * SC)
                    av = avp.tile([VW, 2 * SC], f32, name="av", tag="av")
                    exl = [None] * NKT
                    dense = (p == 0 and qc == 0)
                    for kt in range(NKT + 1):
                        if kt < NKT:
                            kcol = slice(kt * 128, (kt + 1) * 128)
                            st_t = stp.tile([128, 2 * SC], f32, name="st",
                                            tag="st")
                            nc.tensor.matmul(
                                st_t[:, 0:SC], kT_sb[p][0:64, kcol],
                                qT_sb[p][0:64, qcol],
                                start=True, stop=True, tile_position=(0, 0))
                            nc.tensor.matmul(
                                st_t[:, SC:2 * SC], kT_sb[p][64:128, kcol],
                                qT_sb[p][64:128, qcol],
                                start=True, stop=True, tile_position=(64, 0))
                            ex = exs.tile([128, 2 * SC], bf16, name="ex",
                                          tag="ex")
                            nc.scalar.activation(ex[:], st_t[:], EXP)
                            exl[kt] = ex
                        # filler work (front-loaded: slots 0-7)
                        if (kt >= 0 if dense else kt <= 7):
                            if chainq:
                                chainq.pop(0)()
                            elif opq:
                                opq.pop(0)()
                        # finish the previous sweep's normalize once its
                        # reciprocal DMA round-trip has had time to land
                        if kt == 5 and pending_fin:
                            norm_fin(*pending_fin.pop(0))
                        # lagged av consumption
                        for j in AV_SCHED.get(kt, ()):
                            for hh in range(2):
                                h = 2 * p + hh
                                nc.tensor.matmul(
                                    av[:, hh * SC:(hh + 1) * SC],
                                    vx_sb[j][:, h * VW:h * VW + VW],
                                    exl[j][:, hh * SC:(hh + 1) * SC],
                                    start=(j == 0), stop=(j == NKT - 1),
                                    skip_group_check=True)
                    rrf = norm_pre(av)
                    pending_fin.append((av, rrf, p, qc))
            while pending_fin:
                norm_fin(*pending_fin.pop(0))
            while chainq:
                chainq.pop(0)()
        # drain remaining out-projection tiles after the last rep
        while opq:
            opq.pop(0)()

    if fix_waits:
        _fix_multi_waits(nc, mybir)
    return nc


def make_inputs(hidden_states, Wq, bq, Wk, bk, Wv, bv, Wo, bo):
    """Shard + preprocess the full inputs into 8 per-core input maps.
    Shared pieces (per-batch hidden transpose, per-half weight slices)
    are computed once and referenced by both cores that use them."""
    import ml_dtypes
    bf16 = ml_dtypes.bfloat16
    f32 = np.float32

    hidden_states = np.asarray(hidden_states, f32)
    hT = [np.ascontiguousarray(hidden_states[b].T).astype(bf16)
          for b in range(B)]
    halves = []
    for half in range(2):
        hs = slice(half * CLOC, half * CLOC + CLOC)
        halves.append({
            "wqT": np.ascontiguousarray(
                (np.asarray(Wq, f32)[hs] * SCALE).T).astype(bf16),
            "wkT": np.ascontiguousarray(np.asarray(Wk, f32)[hs].T).astype(bf16),
            "wvT": np.ascontiguousarray(np.asarray(Wv, f32)[hs].T).astype(bf16),
            "woT": np.ascontiguousarray(np.asarray(Wo, f32)[:, hs].T).astype(bf16),
            "bq": np.ascontiguousarray(np.asarray(bq, f32)[hs] * SCALE),
            "bk": np.ascontiguousarray(np.asarray(bk, f32)[hs]),
            "bv": np.ascontiguousarray(np.asarray(bv, f32)[hs]),
        })
    return [{"hT": hT[c // 2], **halves[c % 2]} for c in range(NCORES)]


def gather_output(results, bo):
    out = np.empty((B, S, E), np.float32)
    bo = np.asarray(bo, np.float32)
    for b in range(B):
        acc = results[2 * b]["outT"].astype(np.float32) + \
              results[2 * b + 1]["outT"].astype(np.float32)
        out[b] = acc.T + bo
    return out


def _get_runner():
    """Build the Bass program + jitted 8-core executable once; reuse."""
    if "runner" in _CACHE:
        return _CACHE["runner"]
    _get_deps()
    import jax
    import numpy as np
    from jax.sharding import Mesh, PartitionSpec
    from jax.experimental.shard_map import shard_map
    from concourse import bass2jax, mybir

    bass2jax.install_neuronx_cc_hook()
    nc = build_program()

    partition_name = (nc.partition_id_tensor.name
                      if nc.partition_id_tensor else None)
    in_names, out_names, out_avals = [], [], []
    for alloc in nc.m.functions[0].allocations:
        if not isinstance(alloc, mybir.MemoryLocationSet):
            continue
        name = alloc.memorylocations[0].name
        if alloc.kind == "ExternalInput":
            if name != partition_name:
                in_names.append(name)
        elif alloc.kind == "ExternalOutput":
            out_names.append(name)
            out_avals.append(jax.core.ShapedArray(
                tuple(alloc.tensor_shape), mybir.dt.np(alloc.dtype)))
    n_params = len(in_names)
    all_in_names = in_names + out_names
    if partition_name is not None:
        all_in_names = all_in_names + [partition_name]

    def _body(*args):
        operands = list(args)
        if partition_name is not None:
            operands.append(bass2jax.partition_id_tensor())
        outs = bass2jax._bass_exec_p.bind(
            *operands,
            out_avals=tuple(out_avals),
            in_names=tuple(all_in_names),
            out_names=tuple(out_names),
            lowering_input_output_aliases=(),
            sim_require_finite=True,
            sim_require_nnan=True,
            nc=nc,
        )
        return tuple(outs)

    devices = jax.devices()[:NCORES]
    mesh = Mesh(np.asarray(devices), ("core",))
    n_outs = len(out_avals)
    sharded = jax.jit(
        shard_map(
            _body, mesh=mesh,
            in_specs=(PartitionSpec("core"),) * (n_params + n_outs),
            out_specs=(PartitionSpec("core"),) * n_outs,
            check_rep=False,
        ),
        donate_argnums=tuple(range(n_params, n_params + n_outs)),
        keep_unused=True,
    )

    def run(in_maps):
        concat_in = [
            np.concatenate([np.asarray(in_maps[c][nm]) for c in range(NCORES)],
                           axis=0)
            for nm in in_names
        ]
        concat_zeros = [
            np.zeros((NCORES * a.shape[0], *a.shape[1:]), a.dtype)
            for a in out_avals
        ]
        out_arrs = sharded(*concat_in, *concat_zeros)
        return [
            {nm: np.asarray(out_arrs[i]).reshape(NCORES, *out_avals[i].shape)[c]
             for i, nm in enumerate(out_names)}
            for c in range(NCORES)
        ]

    _CACHE["runner"] = (run, sharded, in_names, out_avals)
    return _CACHE["runner"]


def kernel(hidden_states, Wq, bq, Wk, bk, Wv, bv, Wo, bo):
    run = _get_runner()[0]
    in_maps = make_inputs(hidden_states, Wq, bq, Wk, bk, Wv, bv, Wo, bo)
    results = run(in_maps)
    return gather_output(results, bo)


def bench(in_maps, iters=20, pipeline=True):
    """Time repeated device executions with device-resident inputs and a
    non-donating jit (zeros reused). Returns per-iter seconds."""
    import time
    import numpy as np
    import jax
    from jax.sharding import Mesh, NamedSharding, PartitionSpec

    run, sharded, in_names, out_avals = _get_runner()

    devices = jax.devices()[:NCORES]
    mesh = Mesh(np.asarray(devices), ("core",))
    sh = NamedSharding(mesh, PartitionSpec("core"))
    concat_in = [
        np.concatenate([np.asarray(in_maps[c][nm]) for c in range(NCORES)], axis=0)
        for nm in in_names
    ]
    dev_in = [jax.device_put(a, sh) for a in concat_in]
    # zeros are donated (consumed) per execution: pre-stage one set per iter
    znp = [np.zeros((NCORES * a.shape[0], *a.shape[1:]), a.dtype)
           for a in out_avals]
    zsets = [[jax.device_put(z, sh) for z in znp] for _ in range(iters + 1)]

    jax.block_until_ready(sharded(*dev_in, *zsets[-1]))  # warm

    if pipeline:
        t0 = time.perf_counter()
        outs = [sharded(*dev_in, *zsets[i]) for i in range(iters)]
        jax.block_until_ready(outs)
        tot = time.perf_counter() - t0
        return [tot / iters] * iters
    ts = []
    for i in range(iters):
        t0 = time.perf_counter()
        jax.block_until_ready(sharded(*dev_in, *zsets[i]))
        ts.append(time.perf_counter() - t0)
    return ts


if __name__ == "__main__":
    rng = np.random.default_rng(0)
    ins = {
        "hidden_states": rng.standard_normal((B, S, E), np.float32),
        "Wq": rng.standard_normal((E, E), np.float32) * E ** -0.5,
        "bq": rng.standard_normal(E).astype(np.float32) * 0.02,
        "Wk": rng.standard_normal((E, E), np.float32) * E ** -0.5,
        "bk": rng.standard_normal(E).astype(np.float32) * 0.02,
        "Wv": rng.standard_normal((E, E), np.float32) * E ** -0.5,
        "bv": rng.standard_normal(E).astype(np.float32) * 0.02,
        "Wo": rng.standard_normal((E, E), np.float32) * E ** -0.5,
        "bo": rng.standard_normal(E).astype(np.float32) * 0.02,
    }
    out = kernel(**ins)
    print(out.shape, out.dtype, np.abs(out).max())


# revision 15
# speedup vs baseline: 1.3935x; 1.1012x over previous
"""CLIP attention (B=4, S=2048, E=1024, H=16, D=64) on 8 Trainium2 cores.

Sharding: core c handles batch b = c // 2 and heads [ (c%2)*8, (c%2)*8+8 ).
Each core computes its 8 heads' attention plus its partial output
projection (contraction over its 512 local context dims); the host sums
the two partials per batch and adds the output bias.

Per-core dataflow (all activations stored transposed, [feature, seq]):
  hT [E, S]            <- host-pretransposed hidden_states[b], bf16
  qT, kT [512, S]      =  Wq_loc @ hT (+bias, query pre-scaled)   on PE
  v    [S, 512]        =  hT.T @ Wv_loc.T (+bias via bcast add), stored
                          as v_ext tiles [128, 8*65] with a ones column
                          per head (fused softmax denominator)
  ST   [k, q]          =  kT_h.T-slices @ qT_h  (scores, transposed;
                          two heads packed in PE row groups 0-63/64-127)
  P^T  = exp(ST)       on ACT, PSUM -> SBUF bf16 (no max subtraction:
                          scores ~ N(0,1), exp is safe in fp32)
  outT_ext [65, q]     =  V_ext.T @ P^T accumulated over k tiles; row 64
                          is the softmax denominator (ones column)
  CT   [512, S]        =  outT * (1/denom) broadcast
  outT_partial [E, S]  =  Wo_loc^T-slices @ CT  -> DRAM fp32

PSUM (8 banks): score tiles 2 slots x 2 banks, av accumulator 1 slot x
2 banks, projection chains 2 slots x 1 bank. The av slot is single-
buffered; av consumption lags exp by ~6 k-tiles so the previous sweep's
normalize (which reads the av slot) completes before this sweep's first
av matmul needs it.

Normalize path (per sweep): the denominator row av[64,:] is copied to
SBUF (DVE), DMA-reshaped [1,1024]->[128,8] so reciprocal runs on 128
DVE lanes instead of 1 (6.5us -> 0.15us), DMA-flattened back, then
broadcast to 64 partitions by two ones-matmuls into projection-pool
banks, copied to SBUF and multiplied into the context (DVE).

Scheduling: per k-tile slot the emission order is score pair -> filler
(next pair's q/k projection chains, finished chunks' out-projections,
front-loaded into slots 0-7) -> lagged av pairs, so a dependency-stalled
av matmul never head-of-line-blocks filler work in the PE queue.
"""

import numpy as np

B, S, E = 4, 2048, 1024
H, D = 16, 64
SCALE = D ** -0.5
NCORES = 8
HLOC = 8            # heads per core
CLOC = HLOC * D     # 512 local context dims
NHP = HLOC // 2     # 4 head pairs
SC = 512            # seq chunk (matmul moving free dim)
NQC = S // SC       # 4
KT = 128            # k tile rows
NKT = S // KT       # 16
NE = E // 128       # 8 contraction chunks for projections
VW = D + 1          # 65: v columns + fused ones column

# av-consumption schedule: slot k of a sweep consumes these exp tiles'
# AV contributions. Lag ~8 slots gives the previous sweep's normalize
# time to release the single av slot; doubled-up late slots clear the
# backlog before the sweep ends.
AV_SCHED = {6: [0], 7: [1], 8: [2], 9: [3], 10: [4], 11: [5],
            12: [6, 7], 13: [8, 9], 14: [10, 11], 15: [12, 13],
            16: [14, 15]}

_CACHE = {}


def _get_deps():
    import sys
    if "/opt/trn_rl_repo" not in sys.path:
        sys.path.insert(0, "/opt/trn_rl_repo")
    import concourse.bass as bass
    import concourse.mybir as mybir
    import concourse.tile as tile
    return bass, mybir, tile


def _fix_multi_waits(nc, mybir):
    """walrus encodes at most ONE semaphore wait per TPB engine
    instruction. Move surplus waits onto a same-engine Drain inserted just
    before the offending instruction (Drains accept many waits)."""
    for f in nc.m.functions:
        for bb in f.blocks:
            ins = bb.instructions
            if not any(i.sync_info and len(i.sync_info.on_wait) > 1
                       for i in ins):
                continue
            out = []
            for i in ins:
                if i.sync_info and len(i.sync_info.on_wait) > 1:
                    w = list(i.sync_info.on_wait)
                    # a wait on the instruction's OWN processor semaphore
                    # is implied by that processor's FIFO order - drop it
                    own = {u.ant_name for u in i.sync_info.on_update}
                    w2 = [x for x in w if x.ant_name not in own]
                    if not w2:
                        w2 = w[-1:]
                    for j, wj in enumerate(w2[:-1]):
                        d = mybir.InstDrain(
                            name=f"{i.name}_wj{j}", ins=[], outs=[],
                            bass_is_fusable=False)
                        d.engine = i.engine
                        d.sync_info = mybir.SyncInfo(on_wait=[wj], on_update=[])
                        out.append(d)
                    i.sync_info = mybir.SyncInfo(
                        on_wait=w2[-1:], on_update=list(i.sync_info.on_update))
                out.append(i)
            bb.instructions = out


def build_program(fix_waits=True, reps=1):
    """Build the single-core Bass/Tile program (same program on all cores)."""
    bass, mybir, tile = _get_deps()
    from contextlib import ExitStack

    f32 = mybir.dt.float32
    bf16 = mybir.dt.bfloat16
    EXP = mybir.ActivationFunctionType.Exp

    nc = bass.Bass()

    hT_d = nc.declare_dram_parameter("hT", [E, S], bf16, isOutput=False)
    wqT_d = nc.declare_dram_parameter("wqT", [E, CLOC], bf16, isOutput=False)
    wkT_d = nc.declare_dram_parameter("wkT", [E, CLOC], bf16, isOutput=False)
    wvT_d = nc.declare_dram_parameter("wvT", [E, CLOC], bf16, isOutput=False)
    woT_d = nc.declare_dram_parameter("woT", [CLOC, E], bf16, isOutput=False)
    bq_d = nc.declare_dram_parameter("bq", [CLOC], f32, isOutput=False)
    bk_d = nc.declare_dram_parameter("bk", [CLOC], f32, isOutput=False)
    bv_d = nc.declare_dram_parameter("bv", [CLOC], f32, isOutput=False)
    outT_d = nc.declare_dram_parameter("outT", [E, S], f32, isOutput=True)

    add = mybir.AluOpType.add
    mult = mybir.AluOpType.mult

    with tile.TileContext(nc) as tc, ExitStack() as ctx:
        sb = ctx.enter_context(tc.tile_pool(name="persist", bufs=1))

        # ---- persistent SBUF tiles ----
        h_sb = [sb.tile([128, S], bf16, name=f"h{e}", tag=f"h{e}") for e in range(NE)]
        wq_sb = [sb.tile([128, CLOC], bf16, name=f"wq{e}", tag=f"wq{e}") for e in range(NE)]
        wk_sb = [sb.tile([128, CLOC], bf16, name=f"wk{e}", tag=f"wk{e}") for e in range(NE)]
        wv_sb = [sb.tile([128, CLOC], bf16, name=f"wv{e}", tag=f"wv{e}") for e in range(NE)]
        wo_sb = [sb.tile([128, E], bf16, name=f"wo{c}", tag=f"wo{c}") for c in range(4)]
        qT_sb = [sb.tile([128, S], bf16, name=f"qT{p}", tag=f"qT{p}") for p in range(NHP)]
        kT_sb = [sb.tile([128, S], bf16, name=f"kT{p}", tag=f"kT{p}") for p in range(NHP)]
        vx_sb = [sb.tile([128, HLOC * VW], bf16, name=f"vx{t}", tag=f"vx{t}") for t in range(NKT)]
        ct_sb = [sb.tile([128, S], bf16, name=f"ct{p}", tag=f"ct{p}") for p in range(NHP)]
        bq_sb = sb.tile([128, 4], f32, name="bq_sb", tag="bq_sb")
        bk_sb = sb.tile([128, 4], f32, name="bk_sb", tag="bk_sb")
        bv_sb = sb.tile([1, CLOC], f32, name="bv_sb", tag="bv_sb")
        bvb_sb = sb.tile([128, CLOC], f32, name="bvb_sb", tag="bvb_sb")
        ones1 = sb.tile([1, 128], f32, name="ones1", tag="ones1")
        ones64 = sb.tile([1, 64], bf16, name="ones64", tag="ones64")

        # ---- input DMAs, ordered by first use, 128 KB chunks so the
        # 8 hardware DMA queues stream them in parallel ----
        nc.sync.dma_start(out=bv_sb[:], in_=bv_d[:])
        for dtile in range(4):
            r = slice(dtile * 128, (dtile + 1) * 128)
            nc.sync.dma_start(out=bq_sb[:, dtile:dtile + 1], in_=bq_d[r])
            nc.sync.dma_start(out=bk_sb[:, dtile:dtile + 1], in_=bk_d[r])
        for e in range(NE):
            r = slice(e * 128, (e + 1) * 128)
            nc.sync.dma_start(out=wv_sb[e][:], in_=wvT_d[r, :])
        scol0 = slice(0, SC)
        for e in range(NE):
            r = slice(e * 128, (e + 1) * 128)
            nc.sync.dma_start(out=h_sb[e][:, scol0], in_=hT_d[r, scol0])
        for e in range(NE):
            r = slice(e * 128, (e + 1) * 128)
            nc.sync.dma_start(out=wq_sb[e][:], in_=wqT_d[r, :])
        for e in range(NE):
            r = slice(e * 128, (e + 1) * 128)
            nc.sync.dma_start(out=wk_sb[e][:], in_=wkT_d[r, :])
        for sc in range(1, NQC):
            scol = slice(sc * SC, (sc + 1) * SC)
            for e in range(NE):
                r = slice(e * 128, (e + 1) * 128)
                nc.sync.dma_start(out=h_sb[e][:, scol], in_=hT_d[r, scol])
        for c in range(4):
            nc.sync.dma_start(out=wo_sb[c][:], in_=woT_d[c * 128:(c + 1) * 128, :])
        nc.vector.memset(ones1[:], 1.0)
        nc.vector.memset(ones64[:], 1.0)

        # ones columns of v_ext (softmax denominator fusion), set once
        for t in range(NKT):
            for h in range(HLOC):
                nc.vector.memset(vx_sb[t][:, h * VW + D:h * VW + D + 1], 1.0)

        # ---- persistent pools (live across reps) ----
        stp = ctx.enter_context(tc.tile_pool(name="stp", bufs=2, space="PSUM"))
        avp = ctx.enter_context(tc.tile_pool(name="avp", bufs=1, space="PSUM"))
        pjp = ctx.enter_context(tc.tile_pool(name="pjp", bufs=2, space="PSUM"))
        exs = ctx.enter_context(tc.tile_pool(name="exs", bufs=10))
        nrm = ctx.enter_context(tc.tile_pool(name="nrm", bufs=2))
        ost = ctx.enter_context(tc.tile_pool(name="ost", bufs=4))

        _chain_ps = {}
        opq = []            # deferred out-projection tiles (persist across reps)

        def qk_chain(w_sb, b_sb, dst_sb, p, scnk, half=None):
            # half=0/1 emits the first/second 4 accumulation steps
            # (smaller PE bursts); half=None emits the whole chain
            dcol = slice(p * 128, (p + 1) * 128)
            scol = slice(scnk * SC, (scnk + 1) * SC)
            if half in (None, 0):
                ps = pjp.tile([128, SC], f32, name="qkps", tag="pj")
                _chain_ps[(p, scnk, dst_sb is qT_sb)] = ps
            else:
                ps = _chain_ps.pop((p, scnk, dst_sb is qT_sb))
            es = (range(NE) if half is None
                  else range(half * NE // 2, (half + 1) * NE // 2))
            for e in es:
                nc.tensor.matmul(
                    ps[:], w_sb[e][:, dcol], h_sb[e][:, scol],
                    start=(e == 0), stop=(e == NE - 1),
                    skip_group_check=True)
            if half in (None, 1):
                nc.vector.tensor_scalar(
                    dst_sb[p][:, scol], ps[:], b_sb[:, p:p + 1],
                    None, op0=add)

        def v_chain(st):
            ps = pjp.tile([128, CLOC], f32, name="vps", tag="pj")
            for e in range(NE):
                nc.tensor.matmul(
                    ps[:], h_sb[e][:, st * 128:(st + 1) * 128], wv_sb[e][:],
                    start=(e == 0), stop=(e == NE - 1))
            nc.vector.tensor_tensor(
                vx_sb[st][:].rearrange("p (h w) -> p h w", w=VW)[:, :, 0:D],
                ps[:].rearrange("p (h w) -> p h w", w=D),
                bvb_sb[:].rearrange("p (h w) -> p h w", w=D),
                op=add)

        def out_proj_tile(scnk, et):
            scol = slice(scnk * SC, (scnk + 1) * SC)
            erow = slice(et * 128, (et + 1) * 128)
            ps = pjp.tile([128, SC], f32, name="ops", tag="pj")
            for c in range(4):
                nc.tensor.matmul(
                    ps[:], wo_sb[c][:, erow], ct_sb[c][:, scol],
                    start=(c == 0), stop=(c == 3))
            ot = ost.tile([128, SC], f32, name="ot", tag="ot")
            nc.vector.tensor_copy(ot[:], ps[:])
            nc.sync.dma_start(out=outT_d[erow, scol], in_=ot[:])

        def norm_pre(av):
            """Compute rrf = 1/denom, landed on partitions 0 and 32:
            DVE copy of the denominator row, DMA reshape to 128 lanes,
            DVE reciprocal, DMA flatten back (twice)."""
            dn = nrm.tile([1, 2 * SC], f32, name="dn", tag="dn")
            nc.vector.tensor_copy(dn[:], av[64:65, :])
            dn128 = nrm.tile([128, 8], f32, name="dn128", tag="dn128")
            nc.sync.dma_start(out=dn128[:], in_=dn[:])
            rr128 = nrm.tile([128, 8], bf16, name="rr128", tag="rr128")
            with nc.allow_low_precision("1/denom in bf16: <=2^-9 rel"):
                nc.vector.reciprocal(rr128[:], dn128[:])
            rrf = nrm.tile([64, 2 * SC], bf16, name="rrf", tag="rrf")
            nc.sync.dma_start(out=rrf[0:1, :], in_=rr128[:])
            nc.sync.dma_start(out=rrf[32:33, :], in_=rr128[:])
            return rrf

        def norm_fin(av, rrf, p, qc):
            """Broadcast 1/denom to 64 partitions (two group-local DVE
            stream shuffles) and multiply into the context."""
            qcol = slice(qc * SC, (qc + 1) * SC)
            rb = nrm.tile([64, 2 * SC], bf16, name="rb", tag="rb")
            bcast = [0] * 32
            nc.vector.stream_shuffle(rb[0:32, :], rrf[0:32, :], bcast)
            nc.vector.stream_shuffle(rb[32:64, :], rrf[32:64, :], bcast)
            nc.vector.tensor_tensor(
                ct_sb[p][0:64, qcol], av[0:64, 0:SC], rb[:, 0:SC],
                op=mult)
            nc.vector.tensor_tensor(
                ct_sb[p][64:128, qcol], av[0:64, SC:2 * SC],
                rb[:, SC:2 * SC], op=mult)
            if p == NHP - 1:
                for et in range(NE):
                    opq.append((lambda s=qc, e=et: out_proj_tile(s, e)))

        def head_fill():
            # v-bias broadcast (recomputed per rep)
            bb_ps = pjp.tile([128, CLOC], f32, name="bbps", tag="pj")
            nc.tensor.matmul(bb_ps[:], ones1[:], bv_sb[:], start=True,
                             stop=True)
            nc.vector.tensor_copy(bvb_sb[:], bb_ps[:])

        chainq = []
        pending_fin = []
        for _rep in range(reps):
            if _rep == 0:
                # head: v-bias broadcast, the minimum projections the
                # first sweep needs (v tiles 0-3, q/k chunk 0 of pair 0)
                head_fill()
                for st in range(4):
                    v_chain(st)
                qk_chain(wq_sb, bq_sb, qT_sb, 0, 0)
                qk_chain(wk_sb, bk_sb, kT_sb, 0, 0)

                # first sweep's chain work, ordered by first use inside
                # the (p0, qc0) k-tile loop: k-chunks land before their
                # score tiles, v tiles before their AV matmuls
                cq = []
                for s in range(1, NQC):
                    cq.append(
                        (lambda s=s: qk_chain(wk_sb, bk_sb, kT_sb, 0, s)))
                for st in range(4, NKT):
                    cq.append((lambda st=st: v_chain(st)))
                # reorder: k1 first, then v4.., k2 after v6, k3 after v9
                cq = [cq[0], cq[3], cq[4], cq[5],
                      cq[1], cq[6], cq[7], cq[8],
                      cq[2]] + cq[9:]
                for s in range(1, NQC):
                    cq.append(
                        (lambda s=s: qk_chain(wq_sb, bq_sb, qT_sb, 0, s)))
                chainq.extend(cq)
            else:
                # head + pair-0 q/k chains were pre-pulled during the
                # previous rep's last pair; this rep's v chains run as
                # first-sweep fillers (their vx rewrites serialize after
                # the previous rep's final av reads via WAR tracking)
                for st in range(NKT):
                    chainq.append((lambda st=st: v_chain(st)))

            for p in range(NHP):
                if p < NHP - 1:
                    # half-chains for pair p+1, pulled during pair p's
                    # sweeps (two pulls per projection chunk)
                    for s in range(NQC):
                        for (w, b, d) in ((wq_sb, bq_sb, qT_sb),
                                          (wk_sb, bk_sb, kT_sb)):
                            for hf in (0, 1):
                                chainq.append(
                                    (lambda w=w, b=b, d=d, s=s, q=p + 1,
                                     hf=hf: qk_chain(w, b, d, q, s, hf)))
                elif _rep < reps - 1:
                    # pre-pull the NEXT rep's head and pair-0 q/k
                    # projections into this rep's last-pair fillers so
                    # the next rep's first scores fire at the boundary
                    chainq.append(head_fill)
                    for s in range(NQC):
                        chainq.append(
                            (lambda s=s: qk_chain(wk_sb, bk_sb, kT_sb, 0, s)))
                    for s in range(NQC):
                        chainq.append(
                            (lambda s=s: qk_chain(wq_sb, bq_sb, qT_sb, 0, s)))
                for qc in range(NQC):
                    qcol = slice(qc * SC, (qc + 1) * SC)
                    av = avp.tile([VW, 2 * SC], f32, name="av", tag="av")
                    exl = [None] * NKT
                    dense = (p == 0 and qc == 0)
                    for kt in range(NKT + 1):
                        if kt < NKT:
                            kcol = slice(kt * 128, (kt + 1) * 128)
                            st_t = stp.tile([128, 2 * SC], f32, name="st",
                                            tag="st")
                            nc.tensor.matmul(
                                st_t[:, 0:SC], kT_sb[p][0:64, kcol],
                                qT_sb[p][0:64, qcol],
                                start=True, stop=True, tile_position=(0, 0))
                            nc.tensor.matmul(
                                st_t[:, SC:2 * SC], kT_sb[p][64:128, kcol],
                                qT_sb[p][64:128, qcol],
                                start=True, stop=True, tile_position=(64, 0))
                            ex = exs.tile([128, 2 * SC], bf16, name="ex",
                                          tag="ex")
                            nc.scalar.activation(ex[:], st_t[:], EXP)
                            exl[kt] = ex
                        # filler work (front-loaded: slots 0-7)
                        if (kt >= 0 if dense else kt <= 7):
                            if chainq:
                                chainq.pop(0)()
                            elif opq:
                                opq.pop(0)()
                        # finish the previous sweep's normalize once its
                        # reciprocal DMA round-trip has had time to land
                        if kt == 5 and pending_fin:
                            norm_fin(*pending_fin.pop(0))
                        # lagged av consumption
                        for j in AV_SCHED.get(kt, ()):
                            for hh in range(2):
                                h = 2 * p + hh
                                nc.tensor.matmul(
                                    av[:, hh * SC:(hh + 1) * SC],
                                    vx_sb[j][:, h * VW:h * VW + VW],
                                    exl[j][:, hh * SC:(hh + 1) * SC],
                                    start=(j == 0), stop=(j == NKT - 1),
                                    skip_group_check=True)
                    rrf = norm_pre(av)
                    pending_fin.append((av, rrf, p, qc))
        # drain deferred work after the last rep
        while pending_fin:
            norm_fin(*pending_fin.pop(0))
        while chainq:
            chainq.pop(0)()
        while opq:
            opq.pop(0)()

    if fix_waits:
        _fix_multi_waits(nc, mybir)
    return nc


def make_inputs(hidden_states, Wq, bq, Wk, bk, Wv, bv, Wo, bo):
    """Shard + preprocess the full inputs into 8 per-core input maps.
    Shared pieces (per-batch hidden transpose, per-half weight slices)
    are computed once and referenced by both cores that use them."""
    import ml_dtypes
    bf16 = ml_dtypes.bfloat16
    f32 = np.float32

    hidden_states = np.asarray(hidden_states, f32)
    hT = [np.ascontiguousarray(hidden_states[b].T).astype(bf16)
          for b in range(B)]
    halves = []
    for half in range(2):
        hs = slice(half * CLOC, half * CLOC + CLOC)
        halves.append({
            "wqT": np.ascontiguousarray(
                (np.asarray(Wq, f32)[hs] * SCALE).T).astype(bf16),
            "wkT": np.ascontiguousarray(np.asarray(Wk, f32)[hs].T).astype(bf16),
            "wvT": np.ascontiguousarray(np.asarray(Wv, f32)[hs].T).astype(bf16),
            "woT": np.ascontiguousarray(np.asarray(Wo, f32)[:, hs].T).astype(bf16),
            "bq": np.ascontiguousarray(np.asarray(bq, f32)[hs] * SCALE),
            "bk": np.ascontiguousarray(np.asarray(bk, f32)[hs]),
            "bv": np.ascontiguousarray(np.asarray(bv, f32)[hs]),
        })
    return [{"hT": hT[c // 2], **halves[c % 2]} for c in range(NCORES)]


def gather_output(results, bo):
    out = np.empty((B, S, E), np.float32)
    bo = np.asarray(bo, np.float32)
    for b in range(B):
        acc = results[2 * b]["outT"].astype(np.float32) + \
              results[2 * b + 1]["outT"].astype(np.float32)
        out[b] = acc.T + bo
    return out


def _get_runner():
    """Build the Bass program + jitted 8-core executable once; reuse."""
    if "runner" in _CACHE:
        return _CACHE["runner"]
    _get_deps()
    import jax
    import numpy as np
    from jax.sharding import Mesh, PartitionSpec
    from jax.experimental.shard_map import shard_map
    from concourse import bass2jax, mybir

    bass2jax.install_neuronx_cc_hook()
    nc = build_program()

    partition_name = (nc.partition_id_tensor.name
                      if nc.partition_id_tensor else None)
    in_names, out_names, out_avals = [], [], []
    for alloc in nc.m.functions[0].allocations:
        if not isinstance(alloc, mybir.MemoryLocationSet):
            continue
        name = alloc.memorylocations[0].name
        if alloc.kind == "ExternalInput":
            if name != partition_name:
                in_names.append(name)
        elif alloc.kind == "ExternalOutput":
            out_names.append(name)
            out_avals.append(jax.core.ShapedArray(
                tuple(alloc.tensor_shape), mybir.dt.np(alloc.dtype)))
    n_params = len(in_names)
    all_in_names = in_names + out_names
    if partition_name is not None:
        all_in_names = all_in_names + [partition_name]

    def _body(*args):
        operands = list(args)
        if partition_name is not None:
            operands.append(bass2jax.partition_id_tensor())
        outs = bass2jax._bass_exec_p.bind(
            *operands,
            out_avals=tuple(out_avals),
            in_names=tuple(all_in_names),
            out_names=tuple(out_names),
            lowering_input_output_aliases=(),
            sim_require_finite=True,
            sim_require_nnan=True,
            nc=nc,
        )
        return tuple(outs)

    devices = jax.devices()[:NCORES]
    mesh = Mesh(np.asarray(devices), ("core",))
    n_outs = len(out_avals)
    sharded = jax.jit(
        shard_map(
            _body, mesh=mesh,
            in_specs=(PartitionSpec("core"),) * (n_params + n_outs),
            out_specs=(PartitionSpec("core"),) * n_outs,
            check_rep=False,
        ),
        donate_argnums=tuple(range(n_params, n_params + n_outs)),
        keep_unused=True,
    )

    def run(in_maps):
        concat_in = [
            np.concatenate([np.asarray(in_maps[c][nm]) for c in range(NCORES)],
                           axis=0)
            for nm in in_names
        ]
        concat_zeros = [
            np.zeros((NCORES * a.shape[0], *a.shape[1:]), a.dtype)
            for a in out_avals
        ]
        out_arrs = sharded(*concat_in, *concat_zeros)
        return [
            {nm: np.asarray(out_arrs[i]).reshape(NCORES, *out_avals[i].shape)[c]
             for i, nm in enumerate(out_names)}
            for c in range(NCORES)
        ]

    _CACHE["runner"] = (run, sharded, in_names, out_avals)
    return _CACHE["runner"]


def kernel(hidden_states, Wq, bq, Wk, bk, Wv, bv, Wo, bo):
    run = _get_runner()[0]
    in_maps = make_inputs(hidden_states, Wq, bq, Wk, bk, Wv, bv, Wo, bo)
    results = run(in_maps)
    return gather_output(results, bo)


def bench(in_maps, iters=20, pipeline=True):
    """Time repeated device executions with device-resident inputs and a
    non-donating jit (zeros reused). Returns per-iter seconds."""
    import time
    import numpy as np
    import jax
    from jax.sharding import Mesh, NamedSharding, PartitionSpec

    run, sharded, in_names, out_avals = _get_runner()

    devices = jax.devices()[:NCORES]
    mesh = Mesh(np.asarray(devices), ("core",))
    sh = NamedSharding(mesh, PartitionSpec("core"))
    concat_in = [
        np.concatenate([np.asarray(in_maps[c][nm]) for c in range(NCORES)], axis=0)
        for nm in in_names
    ]
    dev_in = [jax.device_put(a, sh) for a in concat_in]
    # zeros are donated (consumed) per execution: pre-stage one set per iter
    znp = [np.zeros((NCORES * a.shape[0], *a.shape[1:]), a.dtype)
           for a in out_avals]
    zsets = [[jax.device_put(z, sh) for z in znp] for _ in range(iters + 1)]

    jax.block_until_ready(sharded(*dev_in, *zsets[-1]))  # warm

    if pipeline:
        t0 = time.perf_counter()
        outs = [sharded(*dev_in, *zsets[i]) for i in range(iters)]
        jax.block_until_ready(outs)
        tot = time.perf_counter() - t0
        return [tot / iters] * iters
    ts = []
    for i in range(iters):
        t0 = time.perf_counter()
        jax.block_until_ready(sharded(*dev_in, *zsets[i]))
        ts.append(time.perf_counter() - t0)
    return ts


if __name__ == "__main__":
    rng = np.random.default_rng(0)
    ins = {
        "hidden_states": rng.standard_normal((B, S, E), np.float32),
        "Wq": rng.standard_normal((E, E), np.float32) * E ** -0.5,
        "bq": rng.standard_normal(E).astype(np.float32) * 0.02,
        "Wk": rng.standard_normal((E, E), np.float32) * E ** -0.5,
        "bk": rng.standard_normal(E).astype(np.float32) * 0.02,
        "Wv": rng.standard_normal((E, E), np.float32) * E ** -0.5,
        "bv": rng.standard_normal(E).astype(np.float32) * 0.02,
        "Wo": rng.standard_normal((E, E), np.float32) * E ** -0.5,
        "bo": rng.standard_normal(E).astype(np.float32) * 0.02,
    }
    out = kernel(**ins)
    print(out.shape, out.dtype, np.abs(out).max())


# revision 28
# speedup vs baseline: 1.4393x; 1.0328x over previous
"""CLIP attention (B=4, S=2048, E=1024, H=16, D=64) on 8 Trainium2 cores.

Sharding: core c handles batch b = c // 2 and heads [ (c%2)*8, (c%2)*8+8 ).
Each core computes its 8 heads' attention plus its partial output
projection (contraction over its 512 local context dims); the host sums
the two partials per batch and adds the output bias.

Per-core dataflow (all activations stored transposed, [feature, seq]):
  hT [E, S]            <- host-pretransposed hidden_states[b], bf16
  qT, kT [512, S]      =  Wq_loc @ hT (+bias, query pre-scaled)   on PE
  v    [S, 512]        =  hT.T @ Wv_loc.T (+bias via bcast add), stored
                          as v_ext tiles [128, 8*65] with a ones column
                          per head (fused softmax denominator)
  ST   [k, q]          =  kT_h.T-slices @ qT_h  (scores, transposed;
                          two heads packed in PE row groups 0-63/64-127)
  P^T  = exp(ST)       on ACT, PSUM -> SBUF bf16 (no max subtraction:
                          scores ~ N(0,1), exp is safe in fp32)
  outT_ext [65, q]     =  V_ext.T @ P^T accumulated over k tiles; row 64
                          is the softmax denominator (ones column)
  CT   [512, S]        =  outT * (1/denom) broadcast
  outT_partial [E, S]  =  Wo_loc^T-slices @ CT  -> DRAM fp32

PSUM (8 banks): score tiles 2 slots x 2 banks, av accumulator 1 slot x
2 banks, projection chains 2 slots x 1 bank. The av slot is single-
buffered; av consumption lags exp by ~6 k-tiles so the previous sweep's
normalize (which reads the av slot) completes before this sweep's first
av matmul needs it.

Normalize path (per sweep): the denominator row av[64,:] is copied to
SBUF (DVE), DMA-reshaped [1,1024]->[128,8] so reciprocal runs on 128
DVE lanes instead of 1 (6.5us -> 0.15us), DMA-flattened back, then
broadcast to 64 partitions by two ones-matmuls into projection-pool
banks, copied to SBUF and multiplied into the context (DVE).

Scheduling: per k-tile slot the emission order is score pair -> filler
(next pair's q/k projection chains, finished chunks' out-projections,
front-loaded into slots 0-7) -> lagged av pairs, so a dependency-stalled
av matmul never head-of-line-blocks filler work in the PE queue.
"""

import numpy as np

B, S, E = 4, 2048, 1024
H, D = 16, 64
SCALE = D ** -0.5
NCORES = 8
HLOC = 8            # heads per core
CLOC = HLOC * D     # 512 local context dims
NHP = HLOC // 2     # 4 head pairs
SC = 512            # seq chunk (matmul moving free dim)
NQC = S // SC       # 4
KT = 128            # k tile rows
NKT = S // KT       # 16
NE = E // 128       # 8 contraction chunks for projections
VW = D + 1          # 65: v columns + fused ones column

# av-consumption schedule: slot k of a sweep consumes these exp tiles'
# AV contributions. Lag ~8 slots gives the previous sweep's normalize
# time to release the single av slot; doubled-up late slots clear the
# backlog before the sweep ends.
AV_SCHED = {6: [0], 7: [1], 8: [2], 9: [3], 10: [4], 11: [5],
            12: [6, 7], 13: [8, 9], 14: [10, 11], 15: [12, 13],
            16: [14, 15]}

_CACHE = {}


def _get_deps():
    import sys
    if "/opt/trn_rl_repo" not in sys.path:
        sys.path.insert(0, "/opt/trn_rl_repo")
    import concourse.bass as bass
    import concourse.mybir as mybir
    import concourse.tile as tile
    return bass, mybir, tile


def _fix_multi_waits(nc, mybir):
    """walrus encodes at most ONE semaphore wait per TPB engine
    instruction. Move surplus waits onto a same-engine Drain inserted just
    before the offending instruction (Drains accept many waits)."""
    for f in nc.m.functions:
        for bb in f.blocks:
            ins = bb.instructions
            if not any(i.sync_info and len(i.sync_info.on_wait) > 1
                       for i in ins):
                continue
            out = []
            for i in ins:
                if i.sync_info and len(i.sync_info.on_wait) > 1:
                    w = list(i.sync_info.on_wait)
                    # a wait on the instruction's OWN processor semaphore
                    # is implied by that processor's FIFO order - drop it
                    own = {u.ant_name for u in i.sync_info.on_update}
                    w2 = [x for x in w if x.ant_name not in own]
                    if not w2:
                        w2 = w[-1:]
                    for j, wj in enumerate(w2[:-1]):
                        d = mybir.InstDrain(
                            name=f"{i.name}_wj{j}", ins=[], outs=[],
                            bass_is_fusable=False)
                        d.engine = i.engine
                        d.sync_info = mybir.SyncInfo(on_wait=[wj], on_update=[])
                        out.append(d)
                    i.sync_info = mybir.SyncInfo(
                        on_wait=w2[-1:], on_update=list(i.sync_info.on_update))
                out.append(i)
            bb.instructions = out


def build_program(fix_waits=True, reps=1):
    """Build the single-core Bass/Tile program (same program on all cores)."""
    bass, mybir, tile = _get_deps()
    from contextlib import ExitStack

    f32 = mybir.dt.float32
    bf16 = mybir.dt.bfloat16
    EXP = mybir.ActivationFunctionType.Exp

    nc = bass.Bass()

    hT_d = nc.declare_dram_parameter("hT", [E, S], bf16, isOutput=False)
    wqT_d = nc.declare_dram_parameter("wqT", [E, CLOC], bf16, isOutput=False)
    wkT_d = nc.declare_dram_parameter("wkT", [E, CLOC], bf16, isOutput=False)
    wvT_d = nc.declare_dram_parameter("wvT", [E, CLOC], bf16, isOutput=False)
    woT_d = nc.declare_dram_parameter("woT", [CLOC, E], bf16, isOutput=False)
    bq_d = nc.declare_dram_parameter("bq", [CLOC], f32, isOutput=False)
    bk_d = nc.declare_dram_parameter("bk", [CLOC], f32, isOutput=False)
    bv_d = nc.declare_dram_parameter("bv", [CLOC], f32, isOutput=False)
    outT_d = nc.declare_dram_parameter("outT", [E, S], f32, isOutput=True)

    add = mybir.AluOpType.add
    mult = mybir.AluOpType.mult

    with tile.TileContext(nc) as tc, ExitStack() as ctx:
        sb = ctx.enter_context(tc.tile_pool(name="persist", bufs=1))

        # ---- persistent SBUF tiles ----
        h_sb = [sb.tile([128, S], bf16, name=f"h{e}", tag=f"h{e}") for e in range(NE)]
        wq_sb = [sb.tile([128, CLOC], bf16, name=f"wq{e}", tag=f"wq{e}") for e in range(NE)]
        wk_sb = [sb.tile([128, CLOC], bf16, name=f"wk{e}", tag=f"wk{e}") for e in range(NE)]
        wv_sb = [sb.tile([128, CLOC], bf16, name=f"wv{e}", tag=f"wv{e}") for e in range(NE)]
        wo_sb = [sb.tile([128, E], bf16, name=f"wo{c}", tag=f"wo{c}") for c in range(4)]
        qT_sb = [sb.tile([128, S], bf16, name=f"qT{p}", tag=f"qT{p}") for p in range(NHP)]
        kT_sb = [sb.tile([128, S], bf16, name=f"kT{p}", tag=f"kT{p}") for p in range(NHP)]
        vx_sb = [sb.tile([128, HLOC * VW], bf16, name=f"vx{t}", tag=f"vx{t}") for t in range(NKT)]
        ct_sb = [sb.tile([128, S], bf16, name=f"ct{p}", tag=f"ct{p}") for p in range(NHP)]
        bq_sb = sb.tile([128, 4], f32, name="bq_sb", tag="bq_sb")
        bk_sb = sb.tile([128, 4], f32, name="bk_sb", tag="bk_sb")
        bv_sb = sb.tile([1, CLOC], f32, name="bv_sb", tag="bv_sb")
        bvb_sb = sb.tile([128, CLOC], f32, name="bvb_sb", tag="bvb_sb")
        ones1 = sb.tile([1, 128], f32, name="ones1", tag="ones1")
        ones64 = sb.tile([1, 64], bf16, name="ones64", tag="ones64")

        # ---- input DMAs, ordered by first use, 128 KB chunks so the
        # 8 hardware DMA queues stream them in parallel ----
        nc.sync.dma_start(out=bv_sb[:], in_=bv_d[:])
        for dtile in range(4):
            r = slice(dtile * 128, (dtile + 1) * 128)
            nc.sync.dma_start(out=bq_sb[:, dtile:dtile + 1], in_=bq_d[r])
            nc.sync.dma_start(out=bk_sb[:, dtile:dtile + 1], in_=bk_d[r])
        for e in range(NE):
            r = slice(e * 128, (e + 1) * 128)
            nc.sync.dma_start(out=wv_sb[e][:], in_=wvT_d[r, :])
        scol0 = slice(0, SC)
        for e in range(NE):
            r = slice(e * 128, (e + 1) * 128)
            nc.sync.dma_start(out=h_sb[e][:, scol0], in_=hT_d[r, scol0])
        for e in range(NE):
            r = slice(e * 128, (e + 1) * 128)
            nc.sync.dma_start(out=wq_sb[e][:], in_=wqT_d[r, :])
        for e in range(NE):
            r = slice(e * 128, (e + 1) * 128)
            nc.sync.dma_start(out=wk_sb[e][:], in_=wkT_d[r, :])
        for sc in range(1, NQC):
            scol = slice(sc * SC, (sc + 1) * SC)
            for e in range(NE):
                r = slice(e * 128, (e + 1) * 128)
                nc.sync.dma_start(out=h_sb[e][:, scol], in_=hT_d[r, scol])
        for c in range(4):
            nc.sync.dma_start(out=wo_sb[c][:], in_=woT_d[c * 128:(c + 1) * 128, :])
        nc.vector.memset(ones1[:], 1.0)
        nc.vector.memset(ones64[:], 1.0)

        # ones columns of v_ext (softmax denominator fusion), set once
        for t in range(NKT):
            for h in range(HLOC):
                nc.vector.memset(vx_sb[t][:, h * VW + D:h * VW + D + 1], 1.0)

        # ---- persistent pools (live across reps) ----
        stp = ctx.enter_context(tc.tile_pool(name="stp", bufs=2, space="PSUM"))
        avp = ctx.enter_context(tc.tile_pool(name="avp", bufs=1, space="PSUM"))
        pjp = ctx.enter_context(tc.tile_pool(name="pjp", bufs=2, space="PSUM"))
        exs = ctx.enter_context(tc.tile_pool(name="exs", bufs=10))
        nrm = ctx.enter_context(tc.tile_pool(name="nrm", bufs=2))
        ost = ctx.enter_context(tc.tile_pool(name="ost", bufs=4))

        _chain_ps = {}
        opq = []            # deferred out-projection tiles (persist across reps)

        def qk_chain(w_sb, b_sb, dst_sb, p, scnk, half=None):
            # half=0/1 emits the first/second 4 accumulation steps
            # (smaller PE bursts); half=None emits the whole chain
            dcol = slice(p * 128, (p + 1) * 128)
            scol = slice(scnk * SC, (scnk + 1) * SC)
            if half in (None, 0):
                ps = pjp.tile([128, SC], f32, name="qkps", tag="pj")
                _chain_ps[(p, scnk, dst_sb is qT_sb)] = ps
            else:
                ps = _chain_ps.pop((p, scnk, dst_sb is qT_sb))
            es = (range(NE) if half is None
                  else range(half * NE // 2, (half + 1) * NE // 2))
            for e in es:
                nc.tensor.matmul(
                    ps[:], w_sb[e][:, dcol], h_sb[e][:, scol],
                    start=(e == 0), stop=(e == NE - 1),
                    skip_group_check=True)
            if half in (None, 1):
                nc.vector.tensor_scalar(
                    dst_sb[p][:, scol], ps[:], b_sb[:, p:p + 1],
                    None, op0=add)

        def v_chain(st):
            ps = pjp.tile([128, CLOC], f32, name="vps", tag="pj")
            for e in range(NE):
                nc.tensor.matmul(
                    ps[:], h_sb[e][:, st * 128:(st + 1) * 128], wv_sb[e][:],
                    start=(e == 0), stop=(e == NE - 1))
            nc.vector.tensor_tensor(
                vx_sb[st][:].rearrange("p (h w) -> p h w", w=VW)[:, :, 0:D],
                ps[:].rearrange("p (h w) -> p h w", w=D),
                bvb_sb[:].rearrange("p (h w) -> p h w", w=D),
                op=add)

        def out_proj_tile(scnk, et):
            scol = slice(scnk * SC, (scnk + 1) * SC)
            erow = slice(et * 128, (et + 1) * 128)
            ps = pjp.tile([128, SC], f32, name="ops", tag="pj")
            for c in range(4):
                nc.tensor.matmul(
                    ps[:], wo_sb[c][:, erow], ct_sb[c][:, scol],
                    start=(c == 0), stop=(c == 3))
            ot = ost.tile([128, SC], f32, name="ot", tag="ot")
            nc.vector.tensor_copy(ot[:], ps[:])
            nc.sync.dma_start(out=outT_d[erow, scol], in_=ot[:])

        def norm_pre(av):
            """Compute rrf = 1/denom, landed on partitions 0 and 32:
            DVE copy of the denominator row, DMA reshape to 128 lanes,
            DVE reciprocal, DMA flatten back (twice)."""
            dn = nrm.tile([1, 2 * SC], f32, name="dn", tag="dn")
            nc.vector.tensor_copy(dn[:], av[64:65, :])
            dn128 = nrm.tile([128, 8], f32, name="dn128", tag="dn128")
            nc.sync.dma_start(out=dn128[:], in_=dn[:])
            rr128 = nrm.tile([128, 8], bf16, name="rr128", tag="rr128")
            with nc.allow_low_precision("1/denom in bf16: <=2^-9 rel"):
                nc.vector.reciprocal(rr128[:], dn128[:])
            rrf = nrm.tile([64, 2 * SC], bf16, name="rrf", tag="rrf")
            nc.sync.dma_start(out=rrf[0:1, :], in_=rr128[:])
            nc.sync.dma_start(out=rrf[32:33, :], in_=rr128[:])
            return rrf

        def norm_fin(av, rrf, p, qc):
            """Broadcast 1/denom to 64 partitions (two group-local DVE
            stream shuffles) and multiply into the context."""
            qcol = slice(qc * SC, (qc + 1) * SC)
            rb = nrm.tile([64, 2 * SC], bf16, name="rb", tag="rb")
            bcast = [0] * 32
            nc.vector.stream_shuffle(rb[0:32, :], rrf[0:32, :], bcast)
            nc.vector.stream_shuffle(rb[32:64, :], rrf[32:64, :], bcast)
            nc.vector.tensor_tensor(
                ct_sb[p][0:64, qcol], av[0:64, 0:SC], rb[:, 0:SC],
                op=mult)
            nc.vector.tensor_tensor(
                ct_sb[p][64:128, qcol], av[0:64, SC:2 * SC],
                rb[:, SC:2 * SC], op=mult)
            if p == NHP - 1:
                for et in range(NE):
                    opq.append((lambda s=qc, e=et: out_proj_tile(s, e)))

        def head_fill():
            # v-bias broadcast (recomputed per rep)
            bb_ps = pjp.tile([128, CLOC], f32, name="bbps", tag="pj")
            nc.tensor.matmul(bb_ps[:], ones1[:], bv_sb[:], start=True,
                             stop=True)
            nc.vector.tensor_copy(bvb_sb[:], bb_ps[:])

        chainq = []
        pending_fin = []
        for _rep in range(reps):
            if _rep == 0:
                # head: v-bias broadcast, the minimum projections the
                # first sweep needs (v tiles 0-3, q/k chunk 0 of pair 0)
                head_fill()
                for st in range(4):
                    v_chain(st)
                qk_chain(wq_sb, bq_sb, qT_sb, 0, 0)
                qk_chain(wk_sb, bk_sb, kT_sb, 0, 0)

                # first sweep's chain work, ordered by first use inside
                # the (p0, qc0) k-tile loop: k-chunks land before their
                # score tiles, v tiles before their AV matmuls
                cq = []
                for s in range(1, NQC):
                    cq.append(
                        (lambda s=s: qk_chain(wk_sb, bk_sb, kT_sb, 0, s)))
                for st in range(4, NKT):
                    cq.append((lambda st=st: v_chain(st)))
                # reorder: k1 first, then v4.., k2 after v6, k3 after v9
                cq = [cq[0], cq[3], cq[4], cq[5],
                      cq[1], cq[6], cq[7], cq[8],
                      cq[2]] + cq[9:]
                for s in range(1, NQC):
                    cq.append(
                        (lambda s=s: qk_chain(wq_sb, bq_sb, qT_sb, 0, s)))
                chainq.extend(cq)
            else:
                # head + pair-0 q/k chains were pre-pulled during the
                # previous rep's last pair; this rep's v chains run as
                # first-sweep fillers (their vx rewrites serialize after
                # the previous rep's final av reads via WAR tracking)
                for st in range(NKT):
                    chainq.append((lambda st=st: v_chain(st)))

            for p in range(NHP):
                if p < NHP - 1:
                    # half-chains for pair p+1, pulled during pair p's
                    # sweeps (two pulls per projection chunk)
                    for s in range(NQC):
                        for (w, b, d) in ((wq_sb, bq_sb, qT_sb),
                                          (wk_sb, bk_sb, kT_sb)):
                            for hf in (0, 1):
                                chainq.append(
                                    (lambda w=w, b=b, d=d, s=s, q=p + 1,
                                     hf=hf: qk_chain(w, b, d, q, s, hf)))
                elif _rep < reps - 1:
                    # pre-pull the NEXT rep's head and pair-0 q/k
                    # projections into this rep's last-pair fillers so
                    # the next rep's first scores fire at the boundary
                    chainq.append(head_fill)
                    for s in range(NQC):
                        chainq.append(
                            (lambda s=s: qk_chain(wk_sb, bk_sb, kT_sb, 0, s)))
                    for s in range(NQC):
                        chainq.append(
                            (lambda s=s: qk_chain(wq_sb, bq_sb, qT_sb, 0, s)))
                for qc in range(NQC):
                    qcol = slice(qc * SC, (qc + 1) * SC)
                    av = avp.tile([VW, 2 * SC], f32, name="av", tag="av")
                    exl = [None] * NKT
                    dense = (p == 0 and qc == 0)
                    for kt in range(NKT + 1):
                        if kt < NKT:
                            kcol = slice(kt * 128, (kt + 1) * 128)
                            st_t = stp.tile([128, 2 * SC], f32, name="st",
                                            tag="st")
                            nc.tensor.matmul(
                                st_t[:, 0:SC], kT_sb[p][0:64, kcol],
                                qT_sb[p][0:64, qcol],
                                start=True, stop=True, tile_position=(0, 0))
                            nc.tensor.matmul(
                                st_t[:, SC:2 * SC], kT_sb[p][64:128, kcol],
                                qT_sb[p][64:128, qcol],
                                start=True, stop=True, tile_position=(64, 0))
                            ex = exs.tile([128, 2 * SC], bf16, name="ex",
                                          tag="ex")
                            nc.scalar.activation(ex[:], st_t[:], EXP)
                            exl[kt] = ex
                        # filler work (front-loaded: slots 0-7)
                        if (kt >= 0 if dense else kt <= 7):
                            if chainq:
                                chainq.pop(0)()
                            elif opq:
                                opq.pop(0)()
                        # finish the previous sweep's normalize once its
                        # reciprocal DMA round-trip has had time to land
                        if kt == 5 and pending_fin:
                            norm_fin(*pending_fin.pop(0))
                        # lagged av consumption
                        for j in AV_SCHED.get(kt, ()):
                            for hh in range(2):
                                h = 2 * p + hh
                                nc.tensor.matmul(
                                    av[:, hh * SC:(hh + 1) * SC],
                                    vx_sb[j][:, h * VW:h * VW + VW],
                                    exl[j][:, hh * SC:(hh + 1) * SC],
                                    start=(j == 0), stop=(j == NKT - 1),
                                    skip_group_check=True)
                    rrf = norm_pre(av)
                    pending_fin.append((av, rrf, p, qc))
        # drain deferred work after the last rep
        while pending_fin:
            norm_fin(*pending_fin.pop(0))
        while chainq:
            chainq.pop(0)()
        while opq:
            opq.pop(0)()

    if fix_waits:
        _fix_multi_waits(nc, mybir)
    return nc


def make_inputs(hidden_states, Wq, bq, Wk, bk, Wv, bv, Wo, bo):
    """Shard + preprocess the full inputs into 8 per-core input maps.
    Shared pieces (per-batch hidden transpose, per-half weight slices)
    are computed once and referenced by both cores that use them."""
    import ml_dtypes
    bf16 = ml_dtypes.bfloat16
    f32 = np.float32

    hidden_states = np.asarray(hidden_states, f32)
    hT = [np.ascontiguousarray(hidden_states[b].T).astype(bf16)
          for b in range(B)]
    halves = []
    for half in range(2):
        hs = slice(half * CLOC, half * CLOC + CLOC)
        halves.append({
            "wqT": np.ascontiguousarray(
                (np.asarray(Wq, f32)[hs] * SCALE).T).astype(bf16),
            "wkT": np.ascontiguousarray(np.asarray(Wk, f32)[hs].T).astype(bf16),
            "wvT": np.ascontiguousarray(np.asarray(Wv, f32)[hs].T).astype(bf16),
            "woT": np.ascontiguousarray(np.asarray(Wo, f32)[:, hs].T).astype(bf16),
            "bq": np.ascontiguousarray(np.asarray(bq, f32)[hs] * SCALE),
            "bk": np.ascontiguousarray(np.asarray(bk, f32)[hs]),
            "bv": np.ascontiguousarray(np.asarray(bv, f32)[hs]),
        })
    return [{"hT": hT[c // 2], **halves[c % 2]} for c in range(NCORES)]


def gather_output(results, bo):
    out = np.empty((B, S, E), np.float32)
    bo = np.asarray(bo, np.float32)
    for b in range(B):
        acc = results[2 * b]["outT"].astype(np.float32) + \
              results[2 * b + 1]["outT"].astype(np.float32)
        out[b] = acc.T + bo
    return out


def _get_runner():
    """Build the Bass program + jitted 8-core executable once; reuse."""
    if "runner" in _CACHE:
        return _CACHE["runner"]
    _get_deps()
    import jax
    import numpy as np
    from jax.sharding import Mesh, PartitionSpec
    from jax.experimental.shard_map import shard_map
    from concourse import bass2jax, mybir

    bass2jax.install_neuronx_cc_hook()
    nc = build_program()

    partition_name = (nc.partition_id_tensor.name
                      if nc.partition_id_tensor else None)
    in_names, out_names, out_avals = [], [], []
    for alloc in nc.m.functions[0].allocations:
        if not isinstance(alloc, mybir.MemoryLocationSet):
            continue
        name = alloc.memorylocations[0].name
        if alloc.kind == "ExternalInput":
            if name != partition_name:
                in_names.append(name)
        elif alloc.kind == "ExternalOutput":
            out_names.append(name)
            out_avals.append(jax.core.ShapedArray(
                tuple(alloc.tensor_shape), mybir.dt.np(alloc.dtype)))
    n_params = len(in_names)
    all_in_names = in_names + out_names
    if partition_name is not None:
        all_in_names = all_in_names + [partition_name]

    def _body(*args):
        operands = list(args)
        if partition_name is not None:
            operands.append(bass2jax.partition_id_tensor())
        outs = bass2jax._bass_exec_p.bind(
            *operands,
            out_avals=tuple(out_avals),
            in_names=tuple(all_in_names),
            out_names=tuple(out_names),
            lowering_input_output_aliases=(),
            sim_require_finite=True,
            sim_require_nnan=True,
            nc=nc,
        )
        return tuple(outs)

    devices = jax.devices()[:NCORES]
    mesh = Mesh(np.asarray(devices), ("core",))
    n_outs = len(out_avals)
    sharded = jax.jit(
        shard_map(
            _body, mesh=mesh,
            in_specs=(PartitionSpec("core"),) * (n_params + n_outs),
            out_specs=(PartitionSpec("core"),) * n_outs,
            check_rep=False,
        ),
        donate_argnums=tuple(range(n_params, n_params + n_outs)),
        keep_unused=True,
    )

    def run(in_maps):
        concat_in = [
            np.concatenate([np.asarray(in_maps[c][nm]) for c in range(NCORES)],
                           axis=0)
            for nm in in_names
        ]
        concat_zeros = [
            np.zeros((NCORES * a.shape[0], *a.shape[1:]), a.dtype)
            for a in out_avals
        ]
        out_arrs = sharded(*concat_in, *concat_zeros)
        return [
            {nm: np.asarray(out_arrs[i]).reshape(NCORES, *out_avals[i].shape)[c]
             for i, nm in enumerate(out_names)}
            for c in range(NCORES)
        ]

    _CACHE["runner"] = (run, sharded, in_names, out_avals)
    return _CACHE["runner"]


def kernel(hidden_states, Wq, bq, Wk, bk, Wv, bv, Wo, bo):
    run = _get_runner()[0]
    in_maps = make_inputs(hidden_states, Wq, bq, Wk, bk, Wv, bv, Wo, bo)
    results = run(in_maps)
    return gather_output(results, bo)


def bench(in_maps, iters=20, pipeline=True):
    """Time repeated device executions with device-resident inputs and a
    non-donating jit (zeros reused). Returns per-iter seconds."""
    import time
    import numpy as np
    import jax
    from jax.sharding import Mesh, NamedSharding, PartitionSpec

    run, sharded, in_names, out_avals = _get_runner()

    devices = jax.devices()[:NCORES]
    mesh = Mesh(np.asarray(devices), ("core",))
    sh = NamedSharding(mesh, PartitionSpec("core"))
    concat_in = [
        np.concatenate([np.asarray(in_maps[c][nm]) for c in range(NCORES)], axis=0)
        for nm in in_names
    ]
    dev_in = [jax.device_put(a, sh) for a in concat_in]
    # zeros are donated (consumed) per execution: pre-stage one set per iter
    znp = [np.zeros((NCORES * a.shape[0], *a.shape[1:]), a.dtype)
           for a in out_avals]
    zsets = [[jax.device_put(z, sh) for z in znp] for _ in range(iters + 1)]

    jax.block_until_ready(sharded(*dev_in, *zsets[-1]))  # warm

    if pipeline:
        t0 = time.perf_counter()
        outs = [sharded(*dev_in, *zsets[i]) for i in range(iters)]
        jax.block_until_ready(outs)
        tot = time.perf_counter() - t0
        return [tot / iters] * iters
    ts = []
    for i in range(iters):
        t0 = time.perf_counter()
        jax.block_until_ready(sharded(*dev_in, *zsets[i]))
        ts.append(time.perf_counter() - t0)
    return ts


if __name__ == "__main__":
    rng = np.random.default_rng(0)
    ins = {
        "hidden_states": rng.standard_normal((B, S, E), np.float32),
        "Wq": rng.standard_normal((E, E), np.float32) * E ** -0.5,
        "bq": rng.standard_normal(E).astype(np.float32) * 0.02,
        "Wk": rng.standard_normal((E, E), np.float32) * E ** -0.5,
        "bk": rng.standard_normal(E).astype(np.float32) * 0.02,
        "Wv": rng.standard_normal((E, E), np.float32) * E ** -0.5,
        "bv": rng.standard_normal(E).astype(np.float32) * 0.02,
        "Wo": rng.standard_normal((E, E), np.float32) * E ** -0.5,
        "bo": rng.standard_normal(E).astype(np.float32) * 0.02,
    }
    out = kernel(**ins)
    print(out.shape, out.dtype, np.abs(out).max())


# revision 29
# speedup vs baseline: 3.0421x; 2.1136x over previous
"""CLIP attention (B=4, S=2048, E=1024, H=16, D=64) on 8 Trainium2 cores.

Sharding: core c handles batch b = c // 2 and heads [ (c%2)*8, (c%2)*8+8 ).
Each core computes its 8 heads' attention plus its partial output
projection (contraction over its 512 local context dims); the host sums
the two partials per batch and adds the output bias.

Per-core dataflow (all activations stored transposed, [feature, seq]):
  hT [E, S]            <- host-pretransposed hidden_states[b], bf16
  qT, kT [512, S]      =  Wq_loc @ hT (+bias, query pre-scaled)   on PE
  v    [S, 512]        =  hT.T @ Wv_loc.T (+bias via bcast add), stored
                          as v_ext tiles [128, 8*65] with a ones column
                          per head (fused softmax denominator)
  ST   [k, q]          =  kT_h.T-slices @ qT_h  (scores, transposed;
                          two heads packed in PE row groups 0-63/64-127)
  P^T  = exp(ST)       on ACT, PSUM -> SBUF bf16 (no max subtraction:
                          scores ~ N(0,1), exp is safe in fp32)
  outT_ext [65, q]     =  V_ext.T @ P^T accumulated over k tiles; row 64
                          is the softmax denominator (ones column)
  CT   [512, S]        =  outT * (1/denom) broadcast
  outT_partial [E, S]  =  Wo_loc^T-slices @ CT  -> DRAM fp32

PSUM (8 banks): score tiles 2 slots x 2 banks, av accumulator 1 slot x
2 banks, projection chains 2 slots x 1 bank. The av slot is single-
buffered; av consumption lags exp by ~6 k-tiles so the previous sweep's
normalize (which reads the av slot) completes before this sweep's first
av matmul needs it.

Normalize path (per sweep): the denominator row av[64,:] is copied to
SBUF (DVE), DMA-reshaped [1,1024]->[128,8] so reciprocal runs on 128
DVE lanes instead of 1 (6.5us -> 0.15us), DMA-flattened back, then
broadcast to 64 partitions by two ones-matmuls into projection-pool
banks, copied to SBUF and multiplied into the context (DVE).

Scheduling: per k-tile slot the emission order is score pair -> filler
(next pair's q/k projection chains, finished chunks' out-projections,
front-loaded into slots 0-7) -> lagged av pairs, so a dependency-stalled
av matmul never head-of-line-blocks filler work in the PE queue.
"""

import numpy as np

B, S, E = 4, 2048, 1024
H, D = 16, 64
SCALE = D ** -0.5
NCORES = 8
HLOC = 8            # heads per core
CLOC = HLOC * D     # 512 local context dims
NHP = HLOC // 2     # 4 head pairs
SC = 512            # seq chunk (matmul moving free dim)
NQC = S // SC       # 4
KT = 128            # k tile rows
NKT = S // KT       # 16
NE = E // 128       # 8 contraction chunks for projections
VW = D + 1          # 65: v columns + fused ones column

# av-consumption schedule: slot k of a sweep consumes these exp tiles'
# AV contributions. Lag ~8 slots gives the previous sweep's normalize
# time to release the single av slot; doubled-up late slots clear the
# backlog before the sweep ends.
AV_SCHED = {6: [0], 7: [1], 8: [2], 9: [3], 10: [4], 11: [5],
            12: [6, 7], 13: [8, 9], 14: [10, 11], 15: [12, 13],
            16: [14, 15]}

_CACHE = {}


def _get_deps():
    import sys
    if "/opt/trn_rl_repo" not in sys.path:
        sys.path.insert(0, "/opt/trn_rl_repo")
    import concourse.bass as bass
    import concourse.mybir as mybir
    import concourse.tile as tile
    return bass, mybir, tile


def _fix_multi_waits(nc, mybir):
    """walrus encodes at most ONE semaphore wait per TPB engine
    instruction. Move surplus waits onto a same-engine Drain inserted just
    before the offending instruction (Drains accept many waits)."""
    for f in nc.m.functions:
        for bb in f.blocks:
            ins = bb.instructions
            if not any(i.sync_info and len(i.sync_info.on_wait) > 1
                       for i in ins):
                continue
            out = []
            for i in ins:
                if i.sync_info and len(i.sync_info.on_wait) > 1:
                    w = list(i.sync_info.on_wait)
                    # a wait on the instruction's OWN processor semaphore
                    # is implied by that processor's FIFO order - drop it
                    own = {u.ant_name for u in i.sync_info.on_update}
                    w2 = [x for x in w if x.ant_name not in own]
                    if not w2:
                        w2 = w[-1:]
                    for j, wj in enumerate(w2[:-1]):
                        d = mybir.InstDrain(
                            name=f"{i.name}_wj{j}", ins=[], outs=[],
                            bass_is_fusable=False)
                        d.engine = i.engine
                        d.sync_info = mybir.SyncInfo(on_wait=[wj], on_update=[])
                        out.append(d)
                    i.sync_info = mybir.SyncInfo(
                        on_wait=w2[-1:], on_update=list(i.sync_info.on_update))
                out.append(i)
            bb.instructions = out


def build_program(fix_waits=True, reps=1):
    """Build the single-core Bass/Tile program (same program on all cores)."""
    bass, mybir, tile = _get_deps()
    from contextlib import ExitStack

    f32 = mybir.dt.float32
    bf16 = mybir.dt.bfloat16
    EXP = mybir.ActivationFunctionType.Exp

    nc = bass.Bass()

    hT_d = nc.declare_dram_parameter("hT", [E, S], bf16, isOutput=False)
    wqT_d = nc.declare_dram_parameter("wqT", [E, CLOC], bf16, isOutput=False)
    wkT_d = nc.declare_dram_parameter("wkT", [E, CLOC], bf16, isOutput=False)
    wvT_d = nc.declare_dram_parameter("wvT", [E, CLOC], bf16, isOutput=False)
    woT_d = nc.declare_dram_parameter("woT", [CLOC, E], bf16, isOutput=False)
    bq_d = nc.declare_dram_parameter("bq", [CLOC], f32, isOutput=False)
    bk_d = nc.declare_dram_parameter("bk", [CLOC], f32, isOutput=False)
    bv_d = nc.declare_dram_parameter("bv", [CLOC], f32, isOutput=False)
    outT_d = nc.declare_dram_parameter("outT", [E, S], f32, isOutput=True)

    add = mybir.AluOpType.add
    mult = mybir.AluOpType.mult

    with tile.TileContext(nc) as tc, ExitStack() as ctx:
        sb = ctx.enter_context(tc.tile_pool(name="persist", bufs=1))

        # ---- persistent SBUF tiles ----
        h_sb = [sb.tile([128, S], bf16, name=f"h{e}", tag=f"h{e}") for e in range(NE)]
        wq_sb = [sb.tile([128, CLOC], bf16, name=f"wq{e}", tag=f"wq{e}") for e in range(NE)]
        wk_sb = [sb.tile([128, CLOC], bf16, name=f"wk{e}", tag=f"wk{e}") for e in range(NE)]
        wv_sb = [sb.tile([128, CLOC], bf16, name=f"wv{e}", tag=f"wv{e}") for e in range(NE)]
        wo_sb = [sb.tile([128, E], bf16, name=f"wo{c}", tag=f"wo{c}") for c in range(4)]
        qT_sb = [sb.tile([128, S], bf16, name=f"qT{p}", tag=f"qT{p}") for p in range(NHP)]
        kT_sb = [sb.tile([128, S], bf16, name=f"kT{p}", tag=f"kT{p}") for p in range(NHP)]
        vx_sb = [sb.tile([128, HLOC * VW], bf16, name=f"vx{t}", tag=f"vx{t}") for t in range(NKT)]
        ct_sb = [sb.tile([128, S], bf16, name=f"ct{p}", tag=f"ct{p}") for p in range(NHP)]
        bq_sb = sb.tile([128, 4], f32, name="bq_sb", tag="bq_sb")
        bk_sb = sb.tile([128, 4], f32, name="bk_sb", tag="bk_sb")
        bv_sb = sb.tile([1, CLOC], f32, name="bv_sb", tag="bv_sb")
        bvb_sb = sb.tile([128, CLOC], f32, name="bvb_sb", tag="bvb_sb")
        ones1 = sb.tile([1, 128], f32, name="ones1", tag="ones1")
        ones64 = sb.tile([1, 64], bf16, name="ones64", tag="ones64")

        # ---- input DMAs, ordered by first use, 128 KB chunks so the
        # 8 hardware DMA queues stream them in parallel ----
        nc.sync.dma_start(out=bv_sb[:], in_=bv_d[:])
        for dtile in range(4):
            r = slice(dtile * 128, (dtile + 1) * 128)
            nc.sync.dma_start(out=bq_sb[:, dtile:dtile + 1], in_=bq_d[r])
            nc.sync.dma_start(out=bk_sb[:, dtile:dtile + 1], in_=bk_d[r])
        for e in range(NE):
            r = slice(e * 128, (e + 1) * 128)
            nc.sync.dma_start(out=wv_sb[e][:], in_=wvT_d[r, :])
        scol0 = slice(0, SC)
        for e in range(NE):
            r = slice(e * 128, (e + 1) * 128)
            nc.sync.dma_start(out=h_sb[e][:, scol0], in_=hT_d[r, scol0])
        for e in range(NE):
            r = slice(e * 128, (e + 1) * 128)
            nc.sync.dma_start(out=wq_sb[e][:], in_=wqT_d[r, :])
        for e in range(NE):
            r = slice(e * 128, (e + 1) * 128)
            nc.sync.dma_start(out=wk_sb[e][:], in_=wkT_d[r, :])
        for sc in range(1, NQC):
            scol = slice(sc * SC, (sc + 1) * SC)
            for e in range(NE):
                r = slice(e * 128, (e + 1) * 128)
                nc.sync.dma_start(out=h_sb[e][:, scol], in_=hT_d[r, scol])
        for c in range(4):
            nc.sync.dma_start(out=wo_sb[c][:], in_=woT_d[c * 128:(c + 1) * 128, :])
        nc.vector.memset(ones1[:], 1.0)
        nc.vector.memset(ones64[:], 1.0)

        # ones columns of v_ext (softmax denominator fusion), set once
        for t in range(NKT):
            for h in range(HLOC):
                nc.vector.memset(vx_sb[t][:, h * VW + D:h * VW + D + 1], 1.0)

        # ---- persistent pools (live across reps) ----
        stp = ctx.enter_context(tc.tile_pool(name="stp", bufs=2, space="PSUM"))
        avp = ctx.enter_context(tc.tile_pool(name="avp", bufs=1, space="PSUM"))
        pjp = ctx.enter_context(tc.tile_pool(name="pjp", bufs=2, space="PSUM"))
        exs = ctx.enter_context(tc.tile_pool(name="exs", bufs=10))
        nrm = ctx.enter_context(tc.tile_pool(name="nrm", bufs=2))
        ost = ctx.enter_context(tc.tile_pool(name="ost", bufs=4))

        _chain_ps = {}
        opq = []            # deferred out-projection tiles (persist across reps)

        def qk_chain(w_sb, b_sb, dst_sb, p, scnk, half=None):
            # half=0/1 emits the first/second 4 accumulation steps
            # (smaller PE bursts); half=None emits the whole chain
            dcol = slice(p * 128, (p + 1) * 128)
            scol = slice(scnk * SC, (scnk + 1) * SC)
            if half in (None, 0):
                ps = pjp.tile([128, SC], f32, name="qkps", tag="pj")
                _chain_ps[(p, scnk, dst_sb is qT_sb)] = ps
            else:
                ps = _chain_ps.pop((p, scnk, dst_sb is qT_sb))
            es = (range(NE) if half is None
                  else range(half * NE // 2, (half + 1) * NE // 2))
            for e in es:
                nc.tensor.matmul(
                    ps[:], w_sb[e][:, dcol], h_sb[e][:, scol],
                    start=(e == 0), stop=(e == NE - 1),
                    skip_group_check=True)
            if half in (None, 1):
                nc.vector.tensor_scalar(
                    dst_sb[p][:, scol], ps[:], b_sb[:, p:p + 1],
                    None, op0=add)

        def v_chain(st):
            ps = pjp.tile([128, CLOC], f32, name="vps", tag="pj")
            for e in range(NE):
                nc.tensor.matmul(
                    ps[:], h_sb[e][:, st * 128:(st + 1) * 128], wv_sb[e][:],
                    start=(e == 0), stop=(e == NE - 1))
            nc.vector.tensor_tensor(
                vx_sb[st][:].rearrange("p (h w) -> p h w", w=VW)[:, :, 0:D],
                ps[:].rearrange("p (h w) -> p h w", w=D),
                bvb_sb[:].rearrange("p (h w) -> p h w", w=D),
                op=add)

        def out_proj_tile(scnk, et):
            scol = slice(scnk * SC, (scnk + 1) * SC)
            erow = slice(et * 128, (et + 1) * 128)
            ps = pjp.tile([128, SC], f32, name="ops", tag="pj")
            for c in range(4):
                nc.tensor.matmul(
                    ps[:], wo_sb[c][:, erow], ct_sb[c][:, scol],
                    start=(c == 0), stop=(c == 3))
            ot = ost.tile([128, SC], f32, name="ot", tag="ot")
            nc.vector.tensor_copy(ot[:], ps[:])
            nc.sync.dma_start(out=outT_d[erow, scol], in_=ot[:])

        def norm_pre(av):
            """Compute rrf = 1/denom, landed on partitions 0 and 32:
            DVE copy of the denominator row, DMA reshape to 128 lanes,
            DVE reciprocal, DMA flatten back (twice)."""
            dn = nrm.tile([1, 2 * SC], f32, name="dn", tag="dn")
            nc.vector.tensor_copy(dn[:], av[64:65, :])
            dn128 = nrm.tile([128, 8], f32, name="dn128", tag="dn128")
            nc.sync.dma_start(out=dn128[:], in_=dn[:])
            rr128 = nrm.tile([128, 8], bf16, name="rr128", tag="rr128")
            with nc.allow_low_precision("1/denom in bf16: <=2^-9 rel"):
                nc.vector.reciprocal(rr128[:], dn128[:])
            rrf = nrm.tile([64, 2 * SC], bf16, name="rrf", tag="rrf")
            nc.sync.dma_start(out=rrf[0:1, :], in_=rr128[:])
            nc.sync.dma_start(out=rrf[32:33, :], in_=rr128[:])
            return rrf

        def norm_fin(av, rrf, p, qc):
            """Broadcast 1/denom to 64 partitions (two group-local DVE
            stream shuffles) and multiply into the context."""
            qcol = slice(qc * SC, (qc + 1) * SC)
            rb = nrm.tile([64, 2 * SC], bf16, name="rb", tag="rb")
            bcast = [0] * 32
            nc.vector.stream_shuffle(rb[0:32, :], rrf[0:32, :], bcast)
            nc.vector.stream_shuffle(rb[32:64, :], rrf[32:64, :], bcast)
            nc.vector.tensor_tensor(
                ct_sb[p][0:64, qcol], av[0:64, 0:SC], rb[:, 0:SC],
                op=mult)
            nc.vector.tensor_tensor(
                ct_sb[p][64:128, qcol], av[0:64, SC:2 * SC],
                rb[:, SC:2 * SC], op=mult)
            if p == NHP - 1:
                for et in range(NE):
                    opq.append((lambda s=qc, e=et: out_proj_tile(s, e)))

        def head_fill():
            # v-bias broadcast (recomputed per rep)
            bb_ps = pjp.tile([128, CLOC], f32, name="bbps", tag="pj")
            nc.tensor.matmul(bb_ps[:], ones1[:], bv_sb[:], start=True,
                             stop=True)
            nc.vector.tensor_copy(bvb_sb[:], bb_ps[:])

        chainq = []
        pending_fin = []
        for _rep in range(reps):
            if _rep == 0:
                # head: v-bias broadcast, the minimum projections the
                # first sweep needs (v tiles 0-3, q/k chunk 0 of pair 0)
                head_fill()
                for st in range(4):
                    v_chain(st)
                qk_chain(wq_sb, bq_sb, qT_sb, 0, 0)
                qk_chain(wk_sb, bk_sb, kT_sb, 0, 0)

                # first sweep's chain work, ordered by first use inside
                # the (p0, qc0) k-tile loop: k-chunks land before their
                # score tiles, v tiles before their AV matmuls
                cq = []
                for s in range(1, NQC):
                    cq.append(
                        (lambda s=s: qk_chain(wk_sb, bk_sb, kT_sb, 0, s)))
                for st in range(4, NKT):
                    cq.append((lambda st=st: v_chain(st)))
                # reorder: k1 first, then v4.., k2 after v6, k3 after v9
                cq = [cq[0], cq[3], cq[4], cq[5],
                      cq[1], cq[6], cq[7], cq[8],
                      cq[2]] + cq[9:]
                for s in range(1, NQC):
                    cq.append(
                        (lambda s=s: qk_chain(wq_sb, bq_sb, qT_sb, 0, s)))
                chainq.extend(cq)
            else:
                # head + pair-0 q/k chains were pre-pulled during the
                # previous rep's last pair; this rep's v chains run as
                # first-sweep fillers (their vx rewrites serialize after
                # the previous rep's final av reads via WAR tracking)
                for st in range(NKT):
                    chainq.append((lambda st=st: v_chain(st)))

            for p in range(NHP):
                if p < NHP - 1:
                    # half-chains for pair p+1, pulled during pair p's
                    # sweeps (two pulls per projection chunk)
                    for s in range(NQC):
                        for (w, b, d) in ((wq_sb, bq_sb, qT_sb),
                                          (wk_sb, bk_sb, kT_sb)):
                            for hf in (0, 1):
                                chainq.append(
                                    (lambda w=w, b=b, d=d, s=s, q=p + 1,
                                     hf=hf: qk_chain(w, b, d, q, s, hf)))
                elif _rep < reps - 1:
                    # pre-pull the NEXT rep's head and pair-0 q/k
                    # projections into this rep's last-pair fillers so
                    # the next rep's first scores fire at the boundary
                    chainq.append(head_fill)
                    for s in range(NQC):
                        chainq.append(
                            (lambda s=s: qk_chain(wk_sb, bk_sb, kT_sb, 0, s)))
                    for s in range(NQC):
                        chainq.append(
                            (lambda s=s: qk_chain(wq_sb, bq_sb, qT_sb, 0, s)))
                for qc in range(NQC):
                    qcol = slice(qc * SC, (qc + 1) * SC)
                    av = avp.tile([VW, 2 * SC], f32, name="av", tag="av")
                    exl = [None] * NKT
                    dense = (p == 0 and qc == 0)
                    for kt in range(NKT + 1):
                        if kt < NKT:
                            kcol = slice(kt * 128, (kt + 1) * 128)
                            st_t = stp.tile([128, 2 * SC], f32, name="st",
                                            tag="st")
                            nc.tensor.matmul(
                                st_t[:, 0:SC], kT_sb[p][0:64, kcol],
                                qT_sb[p][0:64, qcol],
                                start=True, stop=True, tile_position=(0, 0))
                            nc.tensor.matmul(
                                st_t[:, SC:2 * SC], kT_sb[p][64:128, kcol],
                                qT_sb[p][64:128, qcol],
                                start=True, stop=True, tile_position=(64, 0))
                            ex = exs.tile([128, 2 * SC], bf16, name="ex",
                                          tag="ex")
                            nc.scalar.activation(ex[:], st_t[:], EXP)
                            exl[kt] = ex
                        # lagged av consumption first: its deps are long
                        # ready, so it never stalls the PE queue, and it
                        # keeps PE dense right behind the score pair
                        for j in AV_SCHED.get(kt, ()):
                            for hh in range(2):
                                h = 2 * p + hh
                                nc.tensor.matmul(
                                    av[:, hh * SC:(hh + 1) * SC],
                                    vx_sb[j][:, h * VW:h * VW + VW],
                                    exl[j][:, hh * SC:(hh + 1) * SC],
                                    start=(j == 0), stop=(j == NKT - 1),
                                    skip_group_check=True)
                        # finish the previous sweep's normalize once its
                        # reciprocal DMA round-trip has had time to land
                        if kt == 5 and pending_fin:
                            norm_fin(*pending_fin.pop(0))
                        # filler work (front-loaded: slots 0-8)
                        if (kt >= 0 if dense else kt <= 8):
                            if chainq:
                                chainq.pop(0)()
                            elif opq:
                                opq.pop(0)()
                    rrf = norm_pre(av)
                    pending_fin.append((av, rrf, p, qc))
        # drain deferred work after the last rep
        while pending_fin:
            norm_fin(*pending_fin.pop(0))
        while chainq:
            chainq.pop(0)()
        while opq:
            opq.pop(0)()

    if fix_waits:
        _fix_multi_waits(nc, mybir)
    return nc


def make_inputs(hidden_states, Wq, bq, Wk, bk, Wv, bv, Wo, bo):
    """Shard + preprocess the full inputs into 8 per-core input maps.
    Shared pieces (per-batch hidden transpose, per-half weight slices)
    are computed once and referenced by both cores that use them."""
    import ml_dtypes
    bf16 = ml_dtypes.bfloat16
    f32 = np.float32

    hidden_states = np.asarray(hidden_states, f32)
    hT = [np.ascontiguousarray(hidden_states[b].T).astype(bf16)
          for b in range(B)]
    halves = []
    for half in range(2):
        hs = slice(half * CLOC, half * CLOC + CLOC)
        halves.append({
            "wqT": np.ascontiguousarray(
                (np.asarray(Wq, f32)[hs] * SCALE).T).astype(bf16),
            "wkT": np.ascontiguousarray(np.asarray(Wk, f32)[hs].T).astype(bf16),
            "wvT": np.ascontiguousarray(np.asarray(Wv, f32)[hs].T).astype(bf16),
            "woT": np.ascontiguousarray(np.asarray(Wo, f32)[:, hs].T).astype(bf16),
            "bq": np.ascontiguousarray(np.asarray(bq, f32)[hs] * SCALE),
            "bk": np.ascontiguousarray(np.asarray(bk, f32)[hs]),
            "bv": np.ascontiguousarray(np.asarray(bv, f32)[hs]),
        })
    return [{"hT": hT[c // 2], **halves[c % 2]} for c in range(NCORES)]


def gather_output(results, bo):
    out = np.empty((B, S, E), np.float32)
    bo = np.asarray(bo, np.float32)
    for b in range(B):
        acc = results[2 * b]["outT"].astype(np.float32) + \
              results[2 * b + 1]["outT"].astype(np.float32)
        out[b] = acc.T + bo
    return out


def _get_runner():
    """Build the Bass program + jitted 8-core executable once; reuse."""
    if "runner" in _CACHE:
        return _CACHE["runner"]
    _get_deps()
    import jax
    import numpy as np
    from jax.sharding import Mesh, PartitionSpec
    from jax.experimental.shard_map import shard_map
    from concourse import bass2jax, mybir

    bass2jax.install_neuronx_cc_hook()
    nc = build_program()

    partition_name = (nc.partition_id_tensor.name
                      if nc.partition_id_tensor else None)
    in_names, out_names, out_avals = [], [], []
    for alloc in nc.m.functions[0].allocations:
        if not isinstance(alloc, mybir.MemoryLocationSet):
            continue
        name = alloc.memorylocations[0].name
        if alloc.kind == "ExternalInput":
            if name != partition_name:
                in_names.append(name)
        elif alloc.kind == "ExternalOutput":
            out_names.append(name)
            out_avals.append(jax.core.ShapedArray(
                tuple(alloc.tensor_shape), mybir.dt.np(alloc.dtype)))
    n_params = len(in_names)
    all_in_names = in_names + out_names
    if partition_name is not None:
        all_in_names = all_in_names + [partition_name]

    def _body(*args):
        operands = list(args)
        if partition_name is not None:
            operands.append(bass2jax.partition_id_tensor())
        outs = bass2jax._bass_exec_p.bind(
            *operands,
            out_avals=tuple(out_avals),
            in_names=tuple(all_in_names),
            out_names=tuple(out_names),
            lowering_input_output_aliases=(),
            sim_require_finite=True,
            sim_require_nnan=True,
            nc=nc,
        )
        return tuple(outs)

    devices = jax.devices()[:NCORES]
    mesh = Mesh(np.asarray(devices), ("core",))
    n_outs = len(out_avals)
    sharded = jax.jit(
        shard_map(
            _body, mesh=mesh,
            in_specs=(PartitionSpec("core"),) * (n_params + n_outs),
            out_specs=(PartitionSpec("core"),) * n_outs,
            check_rep=False,
        ),
        donate_argnums=tuple(range(n_params, n_params + n_outs)),
        keep_unused=True,
    )

    def run(in_maps):
        concat_in = [
            np.concatenate([np.asarray(in_maps[c][nm]) for c in range(NCORES)],
                           axis=0)
            for nm in in_names
        ]
        concat_zeros = [
            np.zeros((NCORES * a.shape[0], *a.shape[1:]), a.dtype)
            for a in out_avals
        ]
        out_arrs = sharded(*concat_in, *concat_zeros)
        return [
            {nm: np.asarray(out_arrs[i]).reshape(NCORES, *out_avals[i].shape)[c]
             for i, nm in enumerate(out_names)}
            for c in range(NCORES)
        ]

    _CACHE["runner"] = (run, sharded, in_names, out_avals)
    return _CACHE["runner"]


def kernel(hidden_states, Wq, bq, Wk, bk, Wv, bv, Wo, bo):
    run = _get_runner()[0]
    in_maps = make_inputs(hidden_states, Wq, bq, Wk, bk, Wv, bv, Wo, bo)
    results = run(in_maps)
    return gather_output(results, bo)


def bench(in_maps, iters=20, pipeline=True):
    """Time repeated device executions with device-resident inputs and a
    non-donating jit (zeros reused). Returns per-iter seconds."""
    import time
    import numpy as np
    import jax
    from jax.sharding import Mesh, NamedSharding, PartitionSpec

    run, sharded, in_names, out_avals = _get_runner()

    devices = jax.devices()[:NCORES]
    mesh = Mesh(np.asarray(devices), ("core",))
    sh = NamedSharding(mesh, PartitionSpec("core"))
    concat_in = [
        np.concatenate([np.asarray(in_maps[c][nm]) for c in range(NCORES)], axis=0)
        for nm in in_names
    ]
    dev_in = [jax.device_put(a, sh) for a in concat_in]
    # zeros are donated (consumed) per execution: pre-stage one set per iter
    znp = [np.zeros((NCORES * a.shape[0], *a.shape[1:]), a.dtype)
           for a in out_avals]
    zsets = [[jax.device_put(z, sh) for z in znp] for _ in range(iters + 1)]

    jax.block_until_ready(sharded(*dev_in, *zsets[-1]))  # warm

    if pipeline:
        t0 = time.perf_counter()
        outs = [sharded(*dev_in, *zsets[i]) for i in range(iters)]
        jax.block_until_ready(outs)
        tot = time.perf_counter() - t0
        return [tot / iters] * iters
    ts = []
    for i in range(iters):
        t0 = time.perf_counter()
        jax.block_until_ready(sharded(*dev_in, *zsets[i]))
        ts.append(time.perf_counter() - t0)
    return ts


if __name__ == "__main__":
    rng = np.random.default_rng(0)
    ins = {
        "hidden_states": rng.standard_normal((B, S, E), np.float32),
        "Wq": rng.standard_normal((E, E), np.float32) * E ** -0.5,
        "bq": rng.standard_normal(E).astype(np.float32) * 0.02,
        "Wk": rng.standard_normal((E, E), np.float32) * E ** -0.5,
        "bk": rng.standard_normal(E).astype(np.float32) * 0.02,
        "Wv": rng.standard_normal((E, E), np.float32) * E ** -0.5,
        "bv": rng.standard_normal(E).astype(np.float32) * 0.02,
        "Wo": rng.standard_normal((E, E), np.float32) * E ** -0.5,
        "bo": rng.standard_normal(E).astype(np.float32) * 0.02,
    }
    out = kernel(**ins)
    print(out.shape, out.dtype, np.abs(out).max())
